# revision 3
# baseline (speedup 1.0000x reference)
"""MultiHeadedAttention Trainium2 kernel v3 (8 NeuronCores, SPMD).

Sharding: core c -> batch b = c//4, head-group r = c%4 (4 of 16 heads).

v3 vs v2 (137617 ns cost model):
  - QUERY compaction: the same mask that gates keys also gates queries
    (masked-query rows are a host-computed constant). The kernel now only
    processes the ~1046 valid queries (padded to 1152), cutting qproj /
    scores / exp / ctx / wo work by ~44%.
  - ctx in the "output-stationary" orientation: out[q,65] = z_sub^T @
    v_aug, 65-column matmuls (cost model charges by out free size), vs
    512-col in v2.  Softmax denominator still via the ones column.
  - normalization by per-partition scalar (reciprocal of the den column)
    fused into the PSUM->SBUF downcast copy on DVE; the [q,dh] -> [dh,q]
    layout flip for the output projection runs on the (otherwise idle)
    DMA xbar via dma_start_transpose.
  - all operand transposition/blocking on the HOST as in v2; f16 operands.

Per core (np = padded compacted token count, 1152 default):
    kT = wkT^T @ xkT            [256, np]   (dk-major, head-pair blocks)
    qT = wqT^T @ xqT            [256, np]   (compacted queries)
    v  = xvT^T @ wvT            [np, 256] -> v_aug [k-tile, head, 65]
    per (m, j-chunk of 384 q):
      sT = kT_h^T @ qT_h        [128k, 384q] per k-tile, 2 heads
      z  = exp(sT/8)            (f16, SBUF)
      ctx[q,65] += z_sub^T @ v_aug   (PSUM, 6 blocks of 65)
      ctx_q = ctx[:, :64] * recip(ctx[:, 64])   (DVE, f16)
      ctx_dmaj = dma_transpose(ctx_q)           [dh, q]
    outT = woT^T @ ctx_dmaj     [D, np] f16
Host: out[b, valid] = sum_r outT_r^T + bo; masked rows = const row.

Self-contained: hardcodes B=2, S=2048, D=1024, H=16.
"""

import os
import numpy as np
from collections import deque
from contextlib import ExitStack

import concourse.bacc as bacc
import concourse.tile as tile
from concourse import mybir
from concourse.bass_utils import run_bass_kernel_spmd
from concourse.masks import make_identity

F32 = mybir.dt.float32
F16 = mybir.dt.float16
AF = mybir.ActivationFunctionType

B, S, D, H = 2, 2048, 1024, 16
DK = 64                      # head dim
HC = 4                       # heads per core
DH = HC * DK                 # 256, local head width
MB = DH // 128               # 2 partition blocks of local head dims
PD = D // 128                # 8 d-blocks
JW = 384                     # q chunk width (<=512 psum bank, 3 qsubs)
VW = DK + 1                  # 65: v columns + denominator ones column
NCORES = 8

_cache = {}
NKP = 1152                   # padded compacted token count (nk_max=1046)


def _chunks(n, step, first=None):
    c0 = 0
    while c0 < n:
        cw = min(first if (first and c0 == 0) else step, n - c0)
        yield c0, cw
        c0 += cw


def _build_nc(nkp=NKP, with_bv=False):
    NKT = nkp // 128
    JC = list(_chunks(nkp, JW))          # [(c0, cw)] q chunks
    NJ = len(JC)
    nc = bacc.Bacc("TRN2", target_bir_lowering=False, debug=False,
                   num_devices=NCORES)

    xqT_d = nc.dram_tensor("xqT", [D, nkp], F16, kind="ExternalInput").ap()
    xkT_d = nc.dram_tensor("xkT", [D, nkp], F16, kind="ExternalInput").ap()
    xvT_d = nc.dram_tensor("xvT", [D, nkp], F16, kind="ExternalInput").ap()
    wqT_d = nc.dram_tensor("wqT", [128, PD * DH], F16, kind="ExternalInput").ap()
    wkT_d = nc.dram_tensor("wkT", [128, PD * DH], F16, kind="ExternalInput").ap()
    wvT_d = nc.dram_tensor("wvT", [128, PD * DH], F16, kind="ExternalInput").ap()
    woT_d = nc.dram_tensor("woT", [128, MB * D], F16, kind="ExternalInput").ap()
    bq_d = nc.dram_tensor("bq", [128, MB], F32, kind="ExternalInput").ap()
    bk_d = nc.dram_tensor("bk", [128, MB], F32, kind="ExternalInput").ap()
    bvr_d = nc.dram_tensor("bvr", [128, DH], F32, kind="ExternalInput").ap()
    vones_d = nc.dram_tensor("vones", [128, NKT * HC], F32,
                             kind="ExternalInput").ap()
    outT_d = nc.dram_tensor("outT", [D, nkp], F16, kind="ExternalOutput").ap()

    xqT_r = xqT_d.rearrange("(a p) c -> p a c", p=128)   # [128, PD, nkp]
    xkT_r = xkT_d.rearrange("(a p) c -> p a c", p=128)
    xvT_r = xvT_d.rearrange("(a p) c -> p a c", p=128)
    outT_r = outT_d.rearrange("(g p) c -> p g c", p=128)  # [128, 8, nkp]

    with tile.TileContext(nc) as tc, ExitStack() as top:
        const = top.enter_context(tc.tile_pool(name="const", bufs=1))
        ident_f = const.tile([128, 128], F32)
        make_identity(nc, ident_f)
        ident_h = const.tile([128, 128], F16)
        nc.vector.tensor_copy(ident_h[:], ident_f[:])

        # --- input DMAs (merged 3D APs), ordered by deadline ---
        # DMA transfers serialize on the (single-slot) DMA_ENGINES resource,
        # so the stream order IS the arrival order.
        KC = list(_chunks(nkp, 512, first=256))   # kproj/xk chunks
        wk_sb = const.tile([128, PD * DH], F16)
        nc.sync.dma_start(out=wk_sb[:, 0:PD * 128],
                          in_=wkT_d[:, 0:PD * 128])
        bk_sb = const.tile([128, MB], F32)
        nc.sync.dma_start(out=bk_sb[:], in_=bk_d[:, :])
        xk_sb = const.tile([128, PD * nkp], F16)
        xk_v = xk_sb[:].rearrange("p (a c) -> p a c", a=PD)
        nc.sync.dma_start(out=xk_v[:, :, 0:KC[0][1]],
                          in_=xkT_r[:, :, 0:KC[0][1]])
        bq_sb = const.tile([128, MB], F32)
        nc.sync.dma_start(out=bq_sb[:], in_=bq_d[:, :])
        wq_sb = const.tile([128, PD * DH], F16)
        nc.sync.dma_start(out=wq_sb[:, 0:PD * 128],
                          in_=wqT_d[:, 0:PD * 128])
        xq_sb = const.tile([128, PD * nkp], F16)
        xq_v = xq_sb[:].rearrange("p (a c) -> p a c", a=PD)
        nc.sync.dma_start(out=xq_v[:, :, 0:JW], in_=xqT_r[:, :, 0:JW])
        for c0, cw in KC[1:]:
            nc.sync.dma_start(out=xk_v[:, :, c0:c0 + cw],
                              in_=xkT_r[:, :, c0:c0 + cw])
        if nkp > JW:
            nc.sync.dma_start(out=xq_v[:, :, JW:min(2 * JW, nkp)],
                              in_=xqT_r[:, :, JW:min(2 * JW, nkp)])
        nc.sync.dma_start(out=wk_sb[:, PD * 128:2 * PD * 128],
                          in_=wkT_d[:, PD * 128:2 * PD * 128])
        wv_sb = const.tile([128, PD * DH], F16)
        nc.sync.dma_start(out=wv_sb[:], in_=wvT_d[:, :])
        nc.sync.dma_start(out=wq_sb[:, PD * 128:2 * PD * 128],
                          in_=wqT_d[:, PD * 128:2 * PD * 128])
        for c0, cw in _chunks(nkp, JW):
            if c0 < 2 * JW:
                continue
            nc.sync.dma_start(out=xq_v[:, :, c0:c0 + cw],
                              in_=xqT_r[:, :, c0:c0 + cw])
        xv_sb = const.tile([128, PD * nkp], F16)
        xv_v = xv_sb[:].rearrange("p (a c) -> p a c", a=PD)
        vones = const.tile([128, NKT * HC], F32)
        for ci, (c0, cw) in enumerate(_chunks(nkp, 512, first=256)):
            nc.sync.dma_start(out=xv_v[:, :, c0:c0 + cw],
                              in_=xvT_r[:, :, c0:c0 + cw])
            if ci == 0:
                nc.sync.dma_start(out=vones[:], in_=vones_d[:, :])
        bvr = const.tile([128, DH], F32)
        if with_bv:
            nc.sync.dma_start(out=bvr[:], in_=bvr_d[:, :])
        wo_sb = const.tile([128, MB * D], F16)
        nc.sync.dma_start(out=wo_sb[:], in_=woT_d[:, :])

        kT = const.tile([128, MB * nkp], F16)
        qT = const.tile([128, MB * nkp], F16)
        v_aug = const.tile([128, NKT * HC * VW], F16)
        ctx_dmaj = const.tile([128, MB * nkp], F16)
        vag = v_aug[:].rearrange("p (t h c) -> p t h c", t=NKT, h=HC)

        def emit_vones():
            # denominator ones column (validity per k-tile slot); emitted
            # as a filler so the DVE queue isn't blocked on the vones DMA
            nc.vector.tensor_copy(
                vag[:, :, :, DK:DK + 1],
                vones[:].rearrange("p (t h) -> p t h", t=NKT)[:, :, :, None])

        ps_s = top.enter_context(tc.tile_pool(name="ps_s", bufs=2, space="PSUM"))
        ps_c = top.enter_context(tc.tile_pool(name="ps_c", bufs=2, space="PSUM"))
        ps_w = top.enter_context(tc.tile_pool(name="ps_w", bufs=2, space="PSUM"))
        zpool = top.enter_context(tc.tile_pool(name="z", bufs=28))
        nqp = top.enter_context(tc.tile_pool(name="nq", bufs=6))
        rcp = top.enter_context(tc.tile_pool(name="rc", bufs=8))
        outsb = top.enter_context(tc.tile_pool(name="outsb", bufs=3))

        # ---------- emission units ----------
        def emit_kproj(m, c0, cw):
            ps = ps_w.tile([128, 512], F32, tag="psw", name="ps_k")
            for kc in range(PD):
                nc.tensor.matmul(
                    ps[:, 0:cw],
                    lhsT=wk_sb[:, (m * PD + kc) * 128:(m * PD + kc + 1) * 128],
                    rhs=xk_sb[:, nkp * kc + c0:nkp * kc + c0 + cw],
                    start=(kc == 0), stop=(kc == PD - 1))
            nc.vector.tensor_scalar_add(
                kT[:, nkp * m + c0:nkp * m + c0 + cw],
                ps[:, 0:cw], bk_sb[:, m:m + 1])

        def emit_qproj(j, m):
            c0, cw = JC[j]
            ps = ps_w.tile([128, 512], F32, tag="psw", name="ps_q")
            for kc in range(PD):
                nc.tensor.matmul(
                    ps[:, 0:cw],
                    lhsT=wq_sb[:, (m * PD + kc) * 128:(m * PD + kc + 1) * 128],
                    rhs=xq_sb[:, nkp * kc + c0:nkp * kc + c0 + cw],
                    start=(kc == 0), stop=(kc == PD - 1))
            nc.vector.tensor_scalar_add(
                qT[:, nkp * m + c0:nkp * m + c0 + cw],
                ps[:, 0:cw], bq_sb[:, m:m + 1])

        bvr_h = bvr[:].rearrange("p (h c) -> p h c", h=HC)

        def emit_vproj_group(g0):
            gs = min(2, NKT - g0)
            ps = ps_w.tile([128, 512], F32, tag="psw", name="ps_v")
            for s4 in range(gs):
                ii = g0 + s4
                for kc in range(PD):
                    nc.tensor.matmul(
                        ps[:, DH * s4:DH * (s4 + 1)],
                        lhsT=xv_sb[:, nkp * kc + 128 * ii:
                                   nkp * kc + 128 * (ii + 1)],
                        rhs=wv_sb[:, DH * kc:DH * (kc + 1)],
                        start=(kc == 0), stop=(kc == PD - 1))
            for s4 in range(gs):
                ii = g0 + s4
                dst = vag[:, ii, :, 0:DK]
                src = ps[:, DH * s4:DH * (s4 + 1)].rearrange(
                    "p (h c) -> p h c", h=HC)
                if with_bv:
                    nc.vector.tensor_add(dst, src, bvr_h)
                    nc.vector.tensor_scalar_mul(
                        dst, dst, vones[:, HC * ii:HC * ii + 1])
                else:
                    nc.vector.tensor_copy(dst, src)

        saved = {}
        zsave = {}

        # ctx accumulation: one region (t, qs) at a time — start_tensor_calc
        # marks the whole 2KB psum zero-region pending, so interleaved
        # accumulation groups within one bank clobber each other.  All 9
        # z tiles of a loop are kept alive and each region's 9 matmuls run
        # back-to-back (as a filler unit in the next loop).
        def emit_ctx_region(j, m, t, qs):
            c0, cw = JC[j]
            nq = cw // 128
            if (j, m) not in saved:
                saved[(j, m)] = ps_c.tile([128, 512], F32, tag="ctx",
                                          name=f"ctx_{j}_{m}")
            ctx = saved[(j, m)]
            zs = zsave[(j, m)]
            for ic in range(NKT):
                nc.tensor.matmul(
                    ctx[:, ((nq * t) + qs) * VW:((nq * t) + qs) * VW + VW],
                    lhsT=zs[ic][:, cw * t + 128 * qs:cw * t + 128 * (qs + 1)],
                    rhs=vag[:, ic, 2 * m + t, :],
                    start=(ic == 0), stop=(ic == NKT - 1))
            if t == 1 and qs == nq - 1:
                del zsave[(j, m)]

        def emit_norm(j, m, qs):
            ctx = saved[(j, m)]
            c0, cw = JC[j]
            ctx_q = nqp.tile([128, 128], F16, tag="nq", name="ctx_q")
            for t in range(2):
                blk = ((cw // 128) * t + qs) * VW
                r = rcp.tile([128, 1], F32, tag="rc", name="recip")
                nc.vector.reciprocal(r[:], ctx[:, blk + DK:blk + DK + 1])
                nc.vector.tensor_scalar_mul(
                    ctx_q[:, DK * t:DK * (t + 1)],
                    ctx[:, blk:blk + DK], r[:])
            # [q, dh] -> [dh, q] via PE transpose (f16 psum out reuses the
            # psw slot size), DVE copyback; avoids the serialized HWDGE
            tp = ps_w.tile([128, 1024], F16, tag="psw", name="tp")
            nc.tensor.transpose(tp[:, 0:128], ctx_q[:], ident_h[:])
            nc.vector.tensor_copy(
                ctx_dmaj[:, nkp * m + c0 + 128 * qs:
                         nkp * m + c0 + 128 * (qs + 1)],
                tp[:, 0:128])

        osb_hold = {}
        stash = {}

        def emit_wo(j, dd):
            c0, cw = JC[j]
            ps = ps_w.tile([128, 512], F32, tag="psw", name="ps_o")
            for m in range(MB):
                nc.tensor.matmul(
                    ps[:, 0:cw], lhsT=wo_sb[:, m * D + 128 * dd:
                                            m * D + 128 * (dd + 1)],
                    rhs=ctx_dmaj[:, nkp * m + c0:nkp * m + c0 + cw],
                    start=(m == 0), stop=(m == MB - 1))
            _wo_out(j, dd, ps)

        def emit_wo_p1(j, dd):
            c0, cw = JC[j]
            ps = ps_w.tile([128, 512], F32, tag="psw", name="ps_o1")
            nc.tensor.matmul(
                ps[:, 0:cw], lhsT=wo_sb[:, 128 * dd:128 * (dd + 1)],
                rhs=ctx_dmaj[:, c0:c0 + cw], start=True, stop=True)
            o0 = outsb.tile([128, JW], F16, tag="o0", bufs=10, name="o0")
            nc.vector.tensor_copy(o0[:, 0:cw], ps[:, 0:cw])
            stash[(j, dd)] = o0

        def emit_wo_p2(j, dd):
            c0, cw = JC[j]
            ps = ps_w.tile([128, 512], F32, tag="psw", name="ps_o2")
            nc.tensor.matmul(
                ps[:, 0:cw], lhsT=wo_sb[:, D + 128 * dd:D + 128 * (dd + 1)],
                rhs=ctx_dmaj[:, nkp + c0:nkp + c0 + cw],
                start=True, stop=False)
            nc.tensor.matmul(ps[:, 0:cw], lhsT=ident_h[:],
                             rhs=stash.pop((j, dd))[:, 0:cw],
                             start=False, stop=True)
            _wo_out(j, dd, ps, drain=True)

        def _wo_out(j, dd, ps, drain=False):
            c0, cw = JC[j]
            if dd % 4 == 0:
                osb_hold[j] = outsb.tile([128, 4 * JW], F16, tag="osb",
                                         name="osb")
            osb = osb_hold[j]
            half = dd % 4
            eng = nc.scalar if (drain and dd % 2 == 1) else nc.vector
            if eng is nc.scalar:
                eng.copy(osb[:, JW * half:JW * half + cw], ps[:, 0:cw])
            else:
                eng.tensor_copy(osb[:, JW * half:JW * half + cw], ps[:, 0:cw])
            if half == 3:
                nc.sync.dma_start(
                    out=outT_r[:, dd - 3:dd + 1, c0:c0 + cw],
                    in_=osb[:].rearrange("p (g c) -> p g c", g=4)[:, :, 0:cw])

        # ---------- schedule ----------
        # named FIFO of filler units; hard prerequisites enforced by need()
        fillers = deque()
        emitted = set()

        def push(name, fn):
            fillers.append((name, fn))

        def pop_filler():
            if fillers:
                name, fn = fillers.popleft()
                fn()
                emitted.add(name)

        def need(*names):
            while any(n not in emitted for n in names):
                assert fillers, f"missing prerequisite {names}"
                pop_filler()

        kproj_names = {0: [], 1: []}

        def attn_loop(j, m, igate=()):
            need(f"q{j}{m}", *kproj_names[m])
            c0, cw = JC[j]
            zs = zsave.setdefault((j, m), [])
            ig = list(igate)
            for i in range(NKT):
                while ig and ig[0][0] <= i:
                    need(ig.pop(0)[1])
                sps = ps_s.tile([128, 1024], F32, tag="sps", name="sps")
                for t in range(2):
                    nc.tensor.matmul(
                        sps[:, 512 * t:512 * t + cw],
                        lhsT=kT[64 * t:64 * (t + 1),
                                nkp * m + 128 * i:nkp * m + 128 * (i + 1)],
                        rhs=qT[64 * t:64 * (t + 1),
                               nkp * m + c0:nkp * m + c0 + cw],
                        start=True, stop=True)
                zt = zpool.tile([128, 2 * JW], F16, tag="z", name="zt")
                zv = zt[:].rearrange("p (b c) -> p b c", b=2)
                sv = sps[:].rearrange("p (b c) -> p b c", b=2)
                nc.scalar.activation(zv[:, :, 0:cw], sv[:, :, 0:cw],
                                     AF.Exp, scale=0.125)
                zs.append(zt)
                pop_filler()
                popped = 0
                while len(fillers) > (NKT - 1 - i) and popped < 4:
                    pop_filler()
                    popped += 1

        def push_ctx_norm(j, m):
            # interleave: both ctx regions of a q-subtile, then its norm,
            # so transposes start early and spread out
            nq = JC[j][1] // 128
            for qs in range(nq):
                for t in range(2):
                    push(f"c{j}{m}{t}{qs}",
                         lambda j=j, m=m, t=t, qs=qs:
                         emit_ctx_region(j, m, t, qs))
                push(f"n{j}{m}{qs}",
                     lambda j=j, m=m, qs=qs: emit_norm(j, m, qs))

        # startup: kproj m0 chunk 0 + qproj (0,0) emitted directly; the
        # remaining kproj m0 chunks run as fillers inside loop 0, gated
        # per k-tile (their xk chunks stream in during the loop).
        emit_kproj(0, *KC[0])
        emit_qproj(0, 0)
        emitted.add("q00")
        igate0 = []
        for ci, (c0, cw) in enumerate(KC):
            if ci == 0:
                continue
            push(f"k0_{ci}", lambda c0=c0, cw=cw: emit_kproj(0, c0, cw))
            igate0.append((c0 // 128, f"k0_{ci}"))

        # m-outer loop order: (0,0)..(NJ-1,0), (0,1)..(NJ-1,1)
        assert NJ >= 3
        loops = [(j, 0) for j in range(NJ)] + [(j, 1) for j in range(NJ)]
        for li, (j, m) in enumerate(loops):
            if li == 1:
                for ci, (c0, cw) in enumerate(KC):
                    push(f"k1_{ci}",
                         lambda c0=c0, cw=cw: emit_kproj(1, c0, cw))
                    kproj_names[1].append(f"k1_{ci}")
            # ctx+norm of the previous loop (loop 0's delayed one extra
            # loop so the xv stream can land before the vproj units pop)
            if li >= 2:
                push_ctx_norm(*loops[li - 1])
                if li == 2:
                    push_ctx_norm(*loops[0])
            # qproj for the next loop, just in time
            if li + 1 < len(loops) and loops[li + 1] != (0, 0):
                nj, nm = loops[li + 1]
                push(f"q{nj}{nm}",
                     lambda nj=nj, nm=nm: emit_qproj(nj, nm))
            if li == 1:
                for g in range(0, NKT, 2):
                    push(f"v{g}", lambda g=g: emit_vproj_group(g))
                    if g == 0:
                        push("vones", emit_vones)
            # wo(x) once norm(x,1) is queued; last chunk via p1/p2 split
            if li >= 2 and loops[li - 1][1] == 1 and loops[li - 1][0] < NJ - 1:
                x = loops[li - 1][0]
                for dd in range(PD):
                    push(f"wo{x}_{dd}", lambda x=x, dd=dd: emit_wo(x, dd))
            if li == NJ:
                for dd in range(PD):
                    push(f"w1_{dd}", lambda dd=dd: emit_wo_p1(NJ - 1, dd))
            attn_loop(j, m, igate0 if li == 0 else ())
        # drain: last loop's ctx + norm, leftover fillers, wo p2
        push_ctx_norm(*loops[-1])
        while fillers:
            pop_filler()
        for dd in range(PD):
            emit_wo_p2(NJ - 1, dd)

    nc.compile()
    return nc


def _get_nc(nkp=NKP, with_bv=False):
    key = ("nc", nkp, with_bv)
    if key not in _cache:
        _cache[key] = _build_nc(nkp=nkp, with_bv=with_bv)
    return _cache[key]


def _blocked_wT(w, inner, m_major=False):
    """w [out_dim, in_dim] -> lhsT layout [128, (in/128)*out_dim] f16.
    kc-major (default): element [p, out*kc + 128*m + c] = w[128m+c, 128kc+p].
    m-major: element [p, (m*PD_in + kc)*128 + c] = same block, m outer."""
    wT = np.ascontiguousarray(w.T, np.float32)          # [in, out]
    nin, nout = wT.shape
    assert inner == nout
    blk = wT.reshape(nin // 128, 128, nout // 128, 128)   # [kc, p, m, c]
    if m_major:
        out = blk.transpose(1, 2, 0, 3)                   # [p, m, kc, c]
    else:
        out = blk.transpose(1, 0, 2, 3)                   # [p, kc, m, c]
    return np.ascontiguousarray(out.reshape(128, -1)).astype(np.float16)


def _shard_inputs(nkp, query, key, value, mask, wq, bq, wk, bk, wv, bv,
                  wo, bo):
    f16, f32 = np.float16, np.float32
    in_maps = []
    per_b = {}
    for b in range(B):
        maskb = np.ascontiguousarray(mask[b, 0]).astype(np.int32)
        idx = np.flatnonzero(maskb)
        nk = idx.size
        idx_pad = np.zeros(nkp, np.int64)
        idx_pad[:min(nk, nkp)] = idx[:nkp]
        keyc = np.asarray(key[b], f32)[idx_pad]
        valc = np.asarray(value[b], f32)[idx_pad]
        qryc = np.asarray(query[b], f32)[idx_pad]
        keyc[nk:] = 0.0
        valc[nk:] = 0.0
        qryc[nk:] = 0.0
        valid = np.zeros(nkp, f32)
        valid[:nk] = 1.0
        vones = np.repeat(valid.reshape(-1, 128).T[:, :, None], HC,
                          axis=2).reshape(128, -1)
        per_b[b] = {
            "xqT": np.ascontiguousarray(qryc.T).astype(f16),
            "xkT": np.ascontiguousarray(keyc.T).astype(f16),
            "xvT": np.ascontiguousarray(valc.T).astype(f16),
            "vones": np.ascontiguousarray(vones, f32),
        }
    for c in range(NCORES):
        b, r = c // 4, c % 4
        rows = slice(DH * r, DH * (r + 1))
        wq_r = np.asarray(wq, f32)[rows, :]
        wk_r = np.asarray(wk, f32)[rows, :]
        wv_r = np.asarray(wv, f32)[rows, :]
        wo_r = np.asarray(wo, f32)[:, rows]
        in_maps.append({
            **per_b[b],
            "wqT": _blocked_wT(wq_r, DH, m_major=True),
            "wkT": _blocked_wT(wk_r, DH, m_major=True),
            "wvT": _blocked_wT(wv_r, DH),
            "woT": _blocked_wT(wo_r, D),
            "bq": np.ascontiguousarray(
                np.asarray(bq, f32)[rows].reshape(MB, 128).T),
            "bk": np.ascontiguousarray(
                np.asarray(bk, f32)[rows].reshape(MB, 128).T),
            "bvr": np.ascontiguousarray(
                np.broadcast_to(np.asarray(bv, f32)[rows], (128, DH))),
        })
    return in_maps


def kernel(query, key, value, mask, wq, bq, wk, bk, wv, bv, wo, bo,
           _return_bench=False):
    mask = np.asarray(mask)
    nk_max = int(mask.reshape(B, -1).sum(1).max())
    nkp = NKP if nk_max <= NKP else ((nk_max + 127) // 128) * 128
    with_bv = bool(np.any(np.asarray(bv)))
    nc = _get_nc(nkp, with_bv)
    in_maps = _shard_inputs(nkp, np.asarray(query), np.asarray(key),
                            np.asarray(value), mask,
                            np.asarray(wq), np.asarray(bq),
                            np.asarray(wk), np.asarray(bk),
                            np.asarray(wv), np.asarray(bv),
                            np.asarray(wo), np.asarray(bo))
    trace = os.environ.get("KTRACE", "") == "1"
    res = run_bass_kernel_spmd(nc, in_maps, list(range(NCORES)), trace=trace)
    bo = np.asarray(bo, np.float32)
    out = np.empty((B, S, D), np.float32)
    for b in range(B):
        acc = res.results[4 * b]["outT"].astype(np.float32)
        for r in range(1, 4):
            acc += res.results[4 * b + r]["outT"].astype(np.float32)
        maskb = np.asarray(mask[b, 0])
        idx = np.flatnonzero(maskb)
        out[b, idx, :] = acc.T[:idx.size] + bo
        # masked queries: uniform softmax over ALL keys -> constant row
        qmask = maskb == 0
        if qmask.any():
            vmean = np.asarray(value[b], np.float32).mean(0)
            ctx_row = vmean @ np.asarray(wv, np.float32).T + np.asarray(
                bv, np.float32)
            const_row = ctx_row @ np.asarray(wo, np.float32).T + bo
            out[b, qmask, :] = const_row
    if _return_bench:
        return out, res
    return out


# revision 4
# speedup vs baseline: 1.0459x; 1.0459x over previous
"""MultiHeadedAttention Trainium2 kernel v3 (8 NeuronCores, SPMD).

Sharding: core c -> batch b = c//4, head-group r = c%4 (4 of 16 heads).

v3 vs v2 (137617 ns cost model):
  - QUERY compaction: the same mask that gates keys also gates queries
    (masked-query rows are a host-computed constant). The kernel now only
    processes the ~1046 valid queries (padded to 1152), cutting qproj /
    scores / exp / ctx / wo work by ~44%.
  - ctx in the "output-stationary" orientation: out[q,65] = z_sub^T @
    v_aug, 65-column matmuls (cost model charges by out free size), vs
    512-col in v2.  Softmax denominator still via the ones column.
  - normalization by per-partition scalar (reciprocal of the den column)
    fused into the PSUM->SBUF downcast copy on DVE; the [q,dh] -> [dh,q]
    layout flip for the output projection runs on the (otherwise idle)
    DMA xbar via dma_start_transpose.
  - all operand transposition/blocking on the HOST as in v2; f16 operands.

Per core (np = padded compacted token count, 1152 default):
    kT = wkT^T @ xkT            [256, np]   (dk-major, head-pair blocks)
    qT = wqT^T @ xqT            [256, np]   (compacted queries)
    v  = xvT^T @ wvT            [np, 256] -> v_aug [k-tile, head, 65]
    per (m, j-chunk of 384 q):
      sT = kT_h^T @ qT_h        [128k, 384q] per k-tile, 2 heads
      z  = exp(sT/8)            (f16, SBUF)
      ctx[q,65] += z_sub^T @ v_aug   (PSUM, 6 blocks of 65)
      ctx_q = ctx[:, :64] * recip(ctx[:, 64])   (DVE, f16)
      ctx_dmaj = dma_transpose(ctx_q)           [dh, q]
    outT = woT^T @ ctx_dmaj     [D, np] f16
Host: out[b, valid] = sum_r outT_r^T + bo; masked rows = const row.

Self-contained: hardcodes B=2, S=2048, D=1024, H=16.
"""

import os
import numpy as np
from collections import deque
from contextlib import ExitStack

import concourse.bacc as bacc
import concourse.tile as tile
from concourse import mybir
from concourse.bass_utils import run_bass_kernel_spmd
from concourse.masks import make_identity

F32 = mybir.dt.float32
F16 = mybir.dt.float16
AF = mybir.ActivationFunctionType

B, S, D, H = 2, 2048, 1024, 16
DK = 64                      # head dim
HC = 4                       # heads per core
DH = HC * DK                 # 256, local head width
MB = DH // 128               # 2 partition blocks of local head dims
PD = D // 128                # 8 d-blocks
JW = 384                     # q chunk width (<=512 psum bank, 3 qsubs)
VW = DK + 1                  # 65: v columns + denominator ones column
NCORES = 8

_cache = {}
NKP = 1152                   # padded compacted token count (nk_max=1046)


def _chunks(n, step, first=None):
    c0 = 0
    while c0 < n:
        cw = min(first if (first and c0 == 0) else step, n - c0)
        yield c0, cw
        c0 += cw


def _build_nc(nkp=NKP, with_bv=False):
    NKT = nkp // 128
    JC = list(_chunks(nkp, JW))          # [(c0, cw)] q chunks
    NJ = len(JC)
    nc = bacc.Bacc("TRN2", target_bir_lowering=False, debug=False,
                   num_devices=NCORES)

    xqT_d = nc.dram_tensor("xqT", [D, nkp], F16, kind="ExternalInput").ap()
    xkT_d = nc.dram_tensor("xkT", [D, nkp], F16, kind="ExternalInput").ap()
    xvT_d = nc.dram_tensor("xvT", [D, nkp], F16, kind="ExternalInput").ap()
    wqT_d = nc.dram_tensor("wqT", [128, PD * DH], F16, kind="ExternalInput").ap()
    wkT_d = nc.dram_tensor("wkT", [128, PD * DH], F16, kind="ExternalInput").ap()
    wvT_d = nc.dram_tensor("wvT", [128, PD * DH], F16, kind="ExternalInput").ap()
    woT_d = nc.dram_tensor("woT", [128, MB * D], F16, kind="ExternalInput").ap()
    bq_d = nc.dram_tensor("bq", [128, MB], F32, kind="ExternalInput").ap()
    bk_d = nc.dram_tensor("bk", [128, MB], F32, kind="ExternalInput").ap()
    bvr_d = nc.dram_tensor("bvr", [128, DH], F32, kind="ExternalInput").ap()
    vones_d = nc.dram_tensor("vones", [128, NKT * HC], F32,
                             kind="ExternalInput").ap()
    outT_d = nc.dram_tensor("outT", [D, nkp], F16, kind="ExternalOutput").ap()

    xqT_r = xqT_d.rearrange("(a p) c -> p a c", p=128)   # [128, PD, nkp]
    xkT_r = xkT_d.rearrange("(a p) c -> p a c", p=128)
    xvT_r = xvT_d.rearrange("(a p) c -> p a c", p=128)
    outT_r = outT_d.rearrange("(g p) c -> p g c", p=128)  # [128, 8, nkp]

    with tile.TileContext(nc) as tc, ExitStack() as top:
        const = top.enter_context(tc.tile_pool(name="const", bufs=1))
        ident_f = const.tile([128, 128], F32)
        make_identity(nc, ident_f)
        ident_h = const.tile([128, 128], F16)
        nc.vector.tensor_copy(ident_h[:], ident_f[:])

        # --- input DMAs (merged 3D APs), ordered by deadline ---
        # DMA transfers serialize on the (single-slot) DMA_ENGINES resource,
        # so the stream order IS the arrival order.
        KC = list(_chunks(nkp, 512, first=256))   # kproj/xk chunks
        wk_sb = const.tile([128, PD * DH], F16)
        nc.sync.dma_start(out=wk_sb[:, 0:PD * 128],
                          in_=wkT_d[:, 0:PD * 128])
        bk_sb = const.tile([128, MB], F32)
        nc.sync.dma_start(out=bk_sb[:], in_=bk_d[:, :])
        xk_sb = const.tile([128, PD * nkp], F16)
        xk_v = xk_sb[:].rearrange("p (a c) -> p a c", a=PD)
        nc.sync.dma_start(out=xk_v[:, :, 0:KC[0][1]],
                          in_=xkT_r[:, :, 0:KC[0][1]])
        bq_sb = const.tile([128, MB], F32)
        nc.sync.dma_start(out=bq_sb[:], in_=bq_d[:, :])
        wq_sb = const.tile([128, PD * DH], F16)
        nc.sync.dma_start(out=wq_sb[:, 0:PD * 128],
                          in_=wqT_d[:, 0:PD * 128])
        xq_sb = const.tile([128, PD * nkp], F16)
        xq_v = xq_sb[:].rearrange("p (a c) -> p a c", a=PD)
        nc.sync.dma_start(out=xq_v[:, :, 0:JW], in_=xqT_r[:, :, 0:JW])
        for c0, cw in KC[1:]:
            nc.sync.dma_start(out=xk_v[:, :, c0:c0 + cw],
                              in_=xkT_r[:, :, c0:c0 + cw])
        nc.sync.dma_start(out=wk_sb[:, PD * 128:2 * PD * 128],
                          in_=wkT_d[:, PD * 128:2 * PD * 128])
        nc.sync.dma_start(out=wq_sb[:, PD * 128:2 * PD * 128],
                          in_=wqT_d[:, PD * 128:2 * PD * 128])
        if nkp > JW:
            nc.sync.dma_start(out=xq_v[:, :, JW:min(2 * JW, nkp)],
                              in_=xqT_r[:, :, JW:min(2 * JW, nkp)])
        wv_sb = const.tile([128, PD * DH], F16)
        nc.sync.dma_start(out=wv_sb[:], in_=wvT_d[:, :])
        for c0, cw in _chunks(nkp, JW):
            if c0 < 2 * JW:
                continue
            nc.sync.dma_start(out=xq_v[:, :, c0:c0 + cw],
                              in_=xqT_r[:, :, c0:c0 + cw])
        xv_sb = const.tile([128, PD * nkp], F16)
        xv_v = xv_sb[:].rearrange("p (a c) -> p a c", a=PD)
        vones = const.tile([128, NKT * HC], F32)
        for ci, (c0, cw) in enumerate(_chunks(nkp, 512, first=256)):
            nc.sync.dma_start(out=xv_v[:, :, c0:c0 + cw],
                              in_=xvT_r[:, :, c0:c0 + cw])
            if ci == 0:
                nc.sync.dma_start(out=vones[:], in_=vones_d[:, :])
        bvr = const.tile([128, DH], F32)
        if with_bv:
            nc.sync.dma_start(out=bvr[:], in_=bvr_d[:, :])
        wo_sb = const.tile([128, MB * D], F16)
        nc.sync.dma_start(out=wo_sb[:], in_=woT_d[:, :])

        kT = const.tile([128, MB * nkp], F16)
        qT = const.tile([128, MB * nkp], F16)
        v_aug = const.tile([128, NKT * HC * VW], F16)
        ctx_dmaj = const.tile([128, MB * nkp], F16)
        vag = v_aug[:].rearrange("p (t h c) -> p t h c", t=NKT, h=HC)

        def emit_vones():
            # denominator ones column (validity per k-tile slot); emitted
            # as a filler so the DVE queue isn't blocked on the vones DMA
            nc.vector.tensor_copy(
                vag[:, :, :, DK:DK + 1],
                vones[:].rearrange("p (t h) -> p t h", t=NKT)[:, :, :, None])

        ps_s = top.enter_context(tc.tile_pool(name="ps_s", bufs=2, space="PSUM"))
        ps_c = top.enter_context(tc.tile_pool(name="ps_c", bufs=2, space="PSUM"))
        ps_w = top.enter_context(tc.tile_pool(name="ps_w", bufs=2, space="PSUM"))
        zpool = top.enter_context(tc.tile_pool(name="z", bufs=28))
        nqp = top.enter_context(tc.tile_pool(name="nq", bufs=6))
        rcp = top.enter_context(tc.tile_pool(name="rc", bufs=8))
        outsb = top.enter_context(tc.tile_pool(name="outsb", bufs=3))

        # ---------- emission units ----------
        def emit_kproj(m, c0, cw):
            ps = ps_w.tile([128, 512], F32, tag="psw", name="ps_k")
            for kc in range(PD):
                nc.tensor.matmul(
                    ps[:, 0:cw],
                    lhsT=wk_sb[:, (m * PD + kc) * 128:(m * PD + kc + 1) * 128],
                    rhs=xk_sb[:, nkp * kc + c0:nkp * kc + c0 + cw],
                    start=(kc == 0), stop=(kc == PD - 1))
            nc.vector.tensor_scalar_add(
                kT[:, nkp * m + c0:nkp * m + c0 + cw],
                ps[:, 0:cw], bk_sb[:, m:m + 1])

        def emit_qproj(j, m):
            c0, cw = JC[j]
            ps = ps_w.tile([128, 512], F32, tag="psw", name="ps_q")
            for kc in range(PD):
                nc.tensor.matmul(
                    ps[:, 0:cw],
                    lhsT=wq_sb[:, (m * PD + kc) * 128:(m * PD + kc + 1) * 128],
                    rhs=xq_sb[:, nkp * kc + c0:nkp * kc + c0 + cw],
                    start=(kc == 0), stop=(kc == PD - 1))
            nc.vector.tensor_scalar_add(
                qT[:, nkp * m + c0:nkp * m + c0 + cw],
                ps[:, 0:cw], bq_sb[:, m:m + 1])

        bvr_h = bvr[:].rearrange("p (h c) -> p h c", h=HC)

        def emit_vproj_group(g0):
            gs = min(2, NKT - g0)
            ps = ps_w.tile([128, 512], F32, tag="psw", name="ps_v")
            for s4 in range(gs):
                ii = g0 + s4
                for kc in range(PD):
                    nc.tensor.matmul(
                        ps[:, DH * s4:DH * (s4 + 1)],
                        lhsT=xv_sb[:, nkp * kc + 128 * ii:
                                   nkp * kc + 128 * (ii + 1)],
                        rhs=wv_sb[:, DH * kc:DH * (kc + 1)],
                        start=(kc == 0), stop=(kc == PD - 1))
            for s4 in range(gs):
                ii = g0 + s4
                dst = vag[:, ii, :, 0:DK]
                src = ps[:, DH * s4:DH * (s4 + 1)].rearrange(
                    "p (h c) -> p h c", h=HC)
                if with_bv:
                    nc.vector.tensor_add(dst, src, bvr_h)
                    nc.vector.tensor_scalar_mul(
                        dst, dst, vones[:, HC * ii:HC * ii + 1])
                else:
                    nc.vector.tensor_copy(dst, src)

        saved = {}
        zsave = {}

        # ctx accumulation: one region (t, qs) at a time — start_tensor_calc
        # marks the whole 2KB psum zero-region pending, so interleaved
        # accumulation groups within one bank clobber each other.  All 9
        # z tiles of a loop are kept alive and each region's 9 matmuls run
        # back-to-back (as a filler unit in the next loop).
        def emit_ctx_region(j, m, t, qs):
            c0, cw = JC[j]
            nq = cw // 128
            if (j, m) not in saved:
                saved[(j, m)] = ps_c.tile([128, 512], F32, tag="ctx",
                                          name=f"ctx_{j}_{m}")
            ctx = saved[(j, m)]
            zs = zsave[(j, m)]
            for ic in range(NKT):
                nc.tensor.matmul(
                    ctx[:, ((nq * t) + qs) * VW:((nq * t) + qs) * VW + VW],
                    lhsT=zs[ic][:, cw * t + 128 * qs:cw * t + 128 * (qs + 1)],
                    rhs=vag[:, ic, 2 * m + t, :],
                    start=(ic == 0), stop=(ic == NKT - 1))
            if t == 1 and qs == nq - 1:
                del zsave[(j, m)]

        def emit_norm(j, m, qs, split_eng=False):
            ctx = saved[(j, m)]
            c0, cw = JC[j]
            nq = cw // 128
            ctx_q = nqp.tile([128, 128], F16, tag="nq", name="ctx_q")
            for t in range(2):
                blk = (nq * t + qs) * VW
                r = rcp.tile([128, 1], F32, tag="rc", name="recip")
                nc.vector.reciprocal(r[:], ctx[:, blk + DK:blk + DK + 1])
                if split_eng and t == 1:
                    nc.scalar.mul(ctx_q[:, DK * t:DK * (t + 1)],
                                  ctx[:, blk:blk + DK], r[:])
                else:
                    nc.vector.tensor_scalar_mul(
                        ctx_q[:, DK * t:DK * (t + 1)],
                        ctx[:, blk:blk + DK], r[:])
            # [q, dh] -> [dh, q] via PE transpose (f16 psum out reuses the
            # psw slot size), DVE copyback; avoids the serialized HWDGE
            tp = ps_w.tile([128, 1024], F16, tag="psw", name="tp")
            nc.tensor.transpose(tp[:, 0:128], ctx_q[:], ident_h[:])
            nc.vector.tensor_copy(
                ctx_dmaj[:, nkp * m + c0 + 128 * qs:
                         nkp * m + c0 + 128 * (qs + 1)],
                tp[:, 0:128])

        osb_hold = {}
        stash = {}

        def emit_wo(j, dd):
            c0, cw = JC[j]
            ps = ps_w.tile([128, 512], F32, tag="psw", name="ps_o")
            for m in range(MB):
                nc.tensor.matmul(
                    ps[:, 0:cw], lhsT=wo_sb[:, m * D + 128 * dd:
                                            m * D + 128 * (dd + 1)],
                    rhs=ctx_dmaj[:, nkp * m + c0:nkp * m + c0 + cw],
                    start=(m == 0), stop=(m == MB - 1))
            _wo_out(j, dd, ps)

        def emit_wo_p1(j, dd):
            c0, cw = JC[j]
            ps = ps_w.tile([128, 512], F32, tag="psw", name="ps_o1")
            nc.tensor.matmul(
                ps[:, 0:cw], lhsT=wo_sb[:, 128 * dd:128 * (dd + 1)],
                rhs=ctx_dmaj[:, c0:c0 + cw], start=True, stop=True)
            o0 = outsb.tile([128, JW], F16, tag="o0", bufs=10, name="o0")
            nc.vector.tensor_copy(o0[:, 0:cw], ps[:, 0:cw])
            stash[(j, dd)] = o0

        def emit_wo_p2(j, dd):
            c0, cw = JC[j]
            ps = ps_w.tile([128, 512], F32, tag="psw", name="ps_o2")
            nc.tensor.matmul(
                ps[:, 0:cw], lhsT=wo_sb[:, D + 128 * dd:D + 128 * (dd + 1)],
                rhs=ctx_dmaj[:, nkp + c0:nkp + c0 + cw],
                start=True, stop=False)
            nc.tensor.matmul(ps[:, 0:cw], lhsT=ident_h[:],
                             rhs=stash.pop((j, dd))[:, 0:cw],
                             start=False, stop=True)
            _wo_out(j, dd, ps, drain=True)

        def _wo_out(j, dd, ps, drain=False):
            # batch output DMAs in quads (HWDGE is a serialized 625ns/DMA
            # resource); the drain path uses eager pairs instead
            c0, cw = JC[j]
            grp = 2 if drain else 4
            if dd % grp == 0:
                osb_hold[j] = outsb.tile([128, grp * JW], F16, tag="osb",
                                         name="osb")
            osb = osb_hold[j]
            half = dd % grp
            eng = nc.scalar if (drain and dd % 2 == 1) else nc.vector
            if eng is nc.scalar:
                eng.copy(osb[:, JW * half:JW * half + cw], ps[:, 0:cw])
            else:
                eng.tensor_copy(osb[:, JW * half:JW * half + cw], ps[:, 0:cw])
            if half == grp - 1:
                nc.sync.dma_start(
                    out=outT_r[:, dd - grp + 1:dd + 1, c0:c0 + cw],
                    in_=osb[:].rearrange("p (g c) -> p g c",
                                         g=grp)[:, :, 0:cw])

        # ---------- schedule ----------
        # named FIFO of filler units; hard prerequisites enforced by need()
        fillers = deque()
        emitted = set()

        def push(name, fn):
            fillers.append((name, fn))

        def pop_filler():
            if fillers:
                name, fn = fillers.popleft()
                fn()
                emitted.add(name)

        def need(*names):
            while any(n not in emitted for n in names):
                assert fillers, f"missing prerequisite {names}"
                pop_filler()

        kproj_names = {0: [], 1: []}

        def attn_loop(j, m, igate=(), reserve=0):
            need(f"q{j}{m}", *kproj_names[m])
            c0, cw = JC[j]
            zs = zsave.setdefault((j, m), [])
            ig = list(igate)
            for i in range(NKT):
                while ig and ig[0][0] <= i:
                    need(ig.pop(0)[1])
                sps = ps_s.tile([128, 1024], F32, tag="sps", name="sps")
                for t in range(2):
                    nc.tensor.matmul(
                        sps[:, 512 * t:512 * t + cw],
                        lhsT=kT[64 * t:64 * (t + 1),
                                nkp * m + 128 * i:nkp * m + 128 * (i + 1)],
                        rhs=qT[64 * t:64 * (t + 1),
                               nkp * m + c0:nkp * m + c0 + cw],
                        start=True, stop=True)
                zt = zpool.tile([128, 2 * JW], F16, tag="z", name="zt")
                zv = zt[:].rearrange("p (b c) -> p b c", b=2)
                sv = sps[:].rearrange("p (b c) -> p b c", b=2)
                nc.scalar.activation(zv[:, :, 0:cw], sv[:, :, 0:cw],
                                     AF.Exp, scale=0.125)
                zs.append(zt)
                pop_filler()
                popped = 0
                while len(fillers) > (NKT - 1 - i) + reserve and popped < 4:
                    pop_filler()
                    popped += 1

        def push_ctx_norm(j, m, split_eng=False):
            # interleave: both ctx regions of a q-subtile, then its norm,
            # so transposes start early and spread out
            nq = JC[j][1] // 128
            for qs in range(nq):
                for t in range(2):
                    push(f"c{j}{m}{t}{qs}",
                         lambda j=j, m=m, t=t, qs=qs:
                         emit_ctx_region(j, m, t, qs))
                push(f"n{j}{m}{qs}",
                     lambda j=j, m=m, qs=qs, s=split_eng:
                     emit_norm(j, m, qs, split_eng=s))

        # startup: kproj m0 chunk 0 + qproj (0,0) emitted directly; the
        # remaining kproj m0 chunks run as fillers inside loop 0, gated
        # per k-tile (their xk chunks stream in during the loop).
        emit_kproj(0, *KC[0])
        emit_qproj(0, 0)
        emitted.add("q00")
        igate0 = []
        for ci, (c0, cw) in enumerate(KC):
            if ci == 0:
                continue
            push(f"k0_{ci}", lambda c0=c0, cw=cw: emit_kproj(0, c0, cw))
            igate0.append((c0 // 128, f"k0_{ci}"))

        # j-outer loop order: (0,0),(0,1),(1,0),(1,1),... so wo(j) work
        # becomes available for the middle loops, not just the last ones
        assert NJ >= 3
        loops = [(j, m) for j in range(NJ) for m in range(2)]
        for li, (j, m) in enumerate(loops):
            if li == 0:
                for ci, (c0, cw) in enumerate(KC):
                    push(f"k1_{ci}",
                         lambda c0=c0, cw=cw: emit_kproj(1, c0, cw))
                    kproj_names[1].append(f"k1_{ci}")
            # ctx+norm of the previous loop (loop 0's delayed one extra
            # loop so the xv stream can land before the vproj units pop)
            if li >= 2:
                push_ctx_norm(*loops[li - 1])
                if li == 2:
                    push_ctx_norm(*loops[0])
            # qproj for the next loop, just in time
            if li + 1 < len(loops) and loops[li + 1] != (0, 0):
                nj, nm = loops[li + 1]
                push(f"q{nj}{nm}",
                     lambda nj=nj, nm=nm: emit_qproj(nj, nm))
            if li == 1:
                for g in range(0, NKT, 2):
                    push(f"v{g}", lambda g=g: emit_vproj_group(g))
                    if g == 0:
                        push("vones", emit_vones)
            # wo(x) once norm(x,1) is queued; last chunk via p1/p2 split
            if li >= 2 and loops[li - 1][1] == 1 and loops[li - 1][0] < NJ - 1:
                x = loops[li - 1][0]
                for dd in range(PD):
                    push(f"wo{x}_{dd}", lambda x=x, dd=dd: emit_wo(x, dd))
            if li >= 1 and loops[li - 1] == (NJ - 1, 0):
                for dd in range(PD):
                    push(f"w1_{dd}", lambda dd=dd: emit_wo_p1(NJ - 1, dd))
            attn_loop(j, m, igate0 if li == 0 else ())
        # drain: last loop's ctx + norm, leftover fillers, wo p2
        push_ctx_norm(*loops[-1])
        while fillers:
            pop_filler()
        for dd in range(PD):
            emit_wo_p2(NJ - 1, dd)

    nc.compile()
    return nc


def _get_nc(nkp=NKP, with_bv=False):
    key = ("nc", nkp, with_bv)
    if key not in _cache:
        _cache[key] = _build_nc(nkp=nkp, with_bv=with_bv)
    return _cache[key]


def _blocked_wT(w, inner, m_major=False):
    """w [out_dim, in_dim] -> lhsT layout [128, (in/128)*out_dim] f16.
    kc-major (default): element [p, out*kc + 128*m + c] = w[128m+c, 128kc+p].
    m-major: element [p, (m*PD_in + kc)*128 + c] = same block, m outer."""
    wT = np.ascontiguousarray(w.T, np.float32)          # [in, out]
    nin, nout = wT.shape
    assert inner == nout
    blk = wT.reshape(nin // 128, 128, nout // 128, 128)   # [kc, p, m, c]
    if m_major:
        out = blk.transpose(1, 2, 0, 3)                   # [p, m, kc, c]
    else:
        out = blk.transpose(1, 0, 2, 3)                   # [p, kc, m, c]
    return np.ascontiguousarray(out.reshape(128, -1)).astype(np.float16)


def _shard_inputs(nkp, query, key, value, mask, wq, bq, wk, bk, wv, bv,
                  wo, bo):
    f16, f32 = np.float16, np.float32
    in_maps = []
    per_b = {}
    for b in range(B):
        maskb = np.ascontiguousarray(mask[b, 0]).astype(np.int32)
        idx = np.flatnonzero(maskb)
        nk = idx.size
        idx_pad = np.zeros(nkp, np.int64)
        idx_pad[:min(nk, nkp)] = idx[:nkp]
        keyc = np.asarray(key[b], f32)[idx_pad]
        valc = np.asarray(value[b], f32)[idx_pad]
        qryc = np.asarray(query[b], f32)[idx_pad]
        keyc[nk:] = 0.0
        valc[nk:] = 0.0
        qryc[nk:] = 0.0
        valid = np.zeros(nkp, f32)
        valid[:nk] = 1.0
        vones = np.repeat(valid.reshape(-1, 128).T[:, :, None], HC,
                          axis=2).reshape(128, -1)
        per_b[b] = {
            "xqT": np.ascontiguousarray(qryc.T).astype(f16),
            "xkT": np.ascontiguousarray(keyc.T).astype(f16),
            "xvT": np.ascontiguousarray(valc.T).astype(f16),
            "vones": np.ascontiguousarray(vones, f32),
        }
    for c in range(NCORES):
        b, r = c // 4, c % 4
        rows = slice(DH * r, DH * (r + 1))
        wq_r = np.asarray(wq, f32)[rows, :]
        wk_r = np.asarray(wk, f32)[rows, :]
        wv_r = np.asarray(wv, f32)[rows, :]
        wo_r = np.asarray(wo, f32)[:, rows]
        in_maps.append({
            **per_b[b],
            "wqT": _blocked_wT(wq_r, DH, m_major=True),
            "wkT": _blocked_wT(wk_r, DH, m_major=True),
            "wvT": _blocked_wT(wv_r, DH),
            "woT": _blocked_wT(wo_r, D),
            "bq": np.ascontiguousarray(
                np.asarray(bq, f32)[rows].reshape(MB, 128).T),
            "bk": np.ascontiguousarray(
                np.asarray(bk, f32)[rows].reshape(MB, 128).T),
            "bvr": np.ascontiguousarray(
                np.broadcast_to(np.asarray(bv, f32)[rows], (128, DH))),
        })
    return in_maps


def kernel(query, key, value, mask, wq, bq, wk, bk, wv, bv, wo, bo,
           _return_bench=False):
    mask = np.asarray(mask)
    nk_max = int(mask.reshape(B, -1).sum(1).max())
    nkp = NKP if nk_max <= NKP else ((nk_max + 127) // 128) * 128
    with_bv = bool(np.any(np.asarray(bv)))
    nc = _get_nc(nkp, with_bv)
    in_maps = _shard_inputs(nkp, np.asarray(query), np.asarray(key),
                            np.asarray(value), mask,
                            np.asarray(wq), np.asarray(bq),
                            np.asarray(wk), np.asarray(bk),
                            np.asarray(wv), np.asarray(bv),
                            np.asarray(wo), np.asarray(bo))
    trace = os.environ.get("KTRACE", "") == "1"
    res = run_bass_kernel_spmd(nc, in_maps, list(range(NCORES)), trace=trace)
    bo = np.asarray(bo, np.float32)
    out = np.empty((B, S, D), np.float32)
    for b in range(B):
        acc = res.results[4 * b]["outT"].astype(np.float32)
        for r in range(1, 4):
            acc += res.results[4 * b + r]["outT"].astype(np.float32)
        maskb = np.asarray(mask[b, 0])
        idx = np.flatnonzero(maskb)
        out[b, idx, :] = acc.T[:idx.size] + bo
        # masked queries: uniform softmax over ALL keys -> constant row
        qmask = maskb == 0
        if qmask.any():
            vmean = np.asarray(value[b], np.float32).mean(0)
            ctx_row = vmean @ np.asarray(wv, np.float32).T + np.asarray(
                bv, np.float32)
            const_row = ctx_row @ np.asarray(wo, np.float32).T + bo
            out[b, qmask, :] = const_row
    if _return_bench:
        return out, res
    return out


# revision 6
# speedup vs baseline: 1.0494x; 1.0033x over previous
"""MultiHeadedAttention Trainium2 kernel v3 (8 NeuronCores, SPMD).

Sharding: core c -> batch b = c//4, head-group r = c%4 (4 of 16 heads).

v3 vs v2 (137617 ns cost model):
  - QUERY compaction: the same mask that gates keys also gates queries
    (masked-query rows are a host-computed constant). The kernel now only
    processes the ~1046 valid queries (padded to 1152), cutting qproj /
    scores / exp / ctx / wo work by ~44%.
  - ctx in the "output-stationary" orientation: out[q,65] = z_sub^T @
    v_aug, 65-column matmuls (cost model charges by out free size), vs
    512-col in v2.  Softmax denominator still via the ones column.
  - normalization by per-partition scalar (reciprocal of the den column)
    fused into the PSUM->SBUF downcast copy on DVE; the [q,dh] -> [dh,q]
    layout flip for the output projection runs on the (otherwise idle)
    DMA xbar via dma_start_transpose.
  - all operand transposition/blocking on the HOST as in v2; f16 operands.

Per core (np = padded compacted token count, 1152 default):
    kT = wkT^T @ xkT            [256, np]   (dk-major, head-pair blocks)
    qT = wqT^T @ xqT            [256, np]   (compacted queries)
    v  = xvT^T @ wvT            [np, 256] -> v_aug [k-tile, head, 65]
    per (m, j-chunk of 384 q):
      sT = kT_h^T @ qT_h        [128k, 384q] per k-tile, 2 heads
      z  = exp(sT/8)            (f16, SBUF)
      ctx[q,65] += z_sub^T @ v_aug   (PSUM, 6 blocks of 65)
      ctx_q = ctx[:, :64] * recip(ctx[:, 64])   (DVE, f16)
      ctx_dmaj = dma_transpose(ctx_q)           [dh, q]
    outT = woT^T @ ctx_dmaj     [D, np] f16
Host: out[b, valid] = sum_r outT_r^T + bo; masked rows = const row.

Self-contained: hardcodes B=2, S=2048, D=1024, H=16.
"""

import os
import numpy as np
from collections import deque
from contextlib import ExitStack

import concourse.bacc as bacc
import concourse.tile as tile
from concourse import mybir
from concourse.bass_utils import run_bass_kernel_spmd
from concourse.masks import make_identity

F32 = mybir.dt.float32
F16 = mybir.dt.float16
AF = mybir.ActivationFunctionType

B, S, D, H = 2, 2048, 1024, 16
DK = 64                      # head dim
HC = 4                       # heads per core
DH = HC * DK                 # 256, local head width
MB = DH // 128               # 2 partition blocks of local head dims
PD = D // 128                # 8 d-blocks
JW = 384                     # q chunk width (<=512 psum bank, 3 qsubs)
VW = DK + 1                  # 65: v columns + denominator ones column
NCORES = 8

_cache = {}
NKP = 1152                   # padded compacted token count (nk_max=1046)


def _chunks(n, step, first=None):
    c0 = 0
    while c0 < n:
        cw = min(first if (first and c0 == 0) else step, n - c0)
        yield c0, cw
        c0 += cw


def _build_nc(nkp=NKP, with_bv=False):
    NKT = nkp // 128
    JC = list(_chunks(nkp, JW))          # [(c0, cw)] q chunks
    NJ = len(JC)
    nc = bacc.Bacc("TRN2", target_bir_lowering=False, debug=False,
                   num_devices=NCORES)

    xqT_d = nc.dram_tensor("xqT", [D, nkp], F16, kind="ExternalInput").ap()
    xkT_d = nc.dram_tensor("xkT", [D, nkp], F16, kind="ExternalInput").ap()
    xvT_d = nc.dram_tensor("xvT", [D, nkp], F16, kind="ExternalInput").ap()
    wqT_d = nc.dram_tensor("wqT", [128, PD * DH], F16, kind="ExternalInput").ap()
    wkT_d = nc.dram_tensor("wkT", [128, PD * DH], F16, kind="ExternalInput").ap()
    wvT_d = nc.dram_tensor("wvT", [128, PD * DH], F16, kind="ExternalInput").ap()
    woT_d = nc.dram_tensor("woT", [128, MB * D], F16, kind="ExternalInput").ap()
    bq_d = nc.dram_tensor("bq", [128, MB], F32, kind="ExternalInput").ap()
    bk_d = nc.dram_tensor("bk", [128, MB], F32, kind="ExternalInput").ap()
    bvr_d = nc.dram_tensor("bvr", [128, DH], F32, kind="ExternalInput").ap()
    vones_d = nc.dram_tensor("vones", [128, NKT * HC], F32,
                             kind="ExternalInput").ap()
    outT_d = nc.dram_tensor("outT", [D, nkp], F16, kind="ExternalOutput").ap()

    xqT_r = xqT_d.rearrange("(a p) c -> p a c", p=128)   # [128, PD, nkp]
    xkT_r = xkT_d.rearrange("(a p) c -> p a c", p=128)
    xvT_r = xvT_d.rearrange("(a p) c -> p a c", p=128)
    outT_r = outT_d.rearrange("(g p) c -> p g c", p=128)  # [128, 8, nkp]

    with tile.TileContext(nc) as tc, ExitStack() as top:
        const = top.enter_context(tc.tile_pool(name="const", bufs=1))
        ident_f = const.tile([128, 128], F32)
        make_identity(nc, ident_f)
        ident_h = const.tile([128, 128], F16)
        nc.vector.tensor_copy(ident_h[:], ident_f[:])

        # --- input DMAs (merged 3D APs), ordered by deadline ---
        # DMA transfers serialize on the (single-slot) DMA_ENGINES resource,
        # so the stream order IS the arrival order.
        KC = list(_chunks(nkp, 512, first=256))   # kproj/xk chunks
        wk_sb = const.tile([128, PD * DH], F16)
        nc.sync.dma_start(out=wk_sb[:, 0:PD * 128],
                          in_=wkT_d[:, 0:PD * 128])
        bk_sb = const.tile([128, MB], F32)
        nc.sync.dma_start(out=bk_sb[:], in_=bk_d[:, :])
        xk_sb = const.tile([128, PD * nkp], F16)
        xk_v = xk_sb[:].rearrange("p (a c) -> p a c", a=PD)
        nc.sync.dma_start(out=xk_v[:, :, 0:KC[0][1]],
                          in_=xkT_r[:, :, 0:KC[0][1]])
        bq_sb = const.tile([128, MB], F32)
        nc.sync.dma_start(out=bq_sb[:], in_=bq_d[:, :])
        wq_sb = const.tile([128, PD * DH], F16)
        nc.sync.dma_start(out=wq_sb[:, 0:PD * 128],
                          in_=wqT_d[:, 0:PD * 128])
        xq_sb = const.tile([128, PD * nkp], F16)
        xq_v = xq_sb[:].rearrange("p (a c) -> p a c", a=PD)
        nc.sync.dma_start(out=xq_v[:, :, 0:JW], in_=xqT_r[:, :, 0:JW])
        for c0, cw in KC[1:]:
            nc.sync.dma_start(out=xk_v[:, :, c0:c0 + cw],
                              in_=xkT_r[:, :, c0:c0 + cw])
        nc.sync.dma_start(out=wk_sb[:, PD * 128:2 * PD * 128],
                          in_=wkT_d[:, PD * 128:2 * PD * 128])
        nc.sync.dma_start(out=wq_sb[:, PD * 128:2 * PD * 128],
                          in_=wqT_d[:, PD * 128:2 * PD * 128])
        if nkp > JW:
            nc.sync.dma_start(out=xq_v[:, :, JW:min(2 * JW, nkp)],
                              in_=xqT_r[:, :, JW:min(2 * JW, nkp)])
        wv_sb = const.tile([128, PD * DH], F16)
        nc.sync.dma_start(out=wv_sb[:], in_=wvT_d[:, :])
        for c0, cw in _chunks(nkp, JW):
            if c0 < 2 * JW:
                continue
            nc.sync.dma_start(out=xq_v[:, :, c0:c0 + cw],
                              in_=xqT_r[:, :, c0:c0 + cw])
        xv_sb = const.tile([128, PD * nkp], F16)
        xv_v = xv_sb[:].rearrange("p (a c) -> p a c", a=PD)
        vones = const.tile([128, NKT * HC], F32)
        for ci, (c0, cw) in enumerate(_chunks(nkp, 512, first=256)):
            nc.sync.dma_start(out=xv_v[:, :, c0:c0 + cw],
                              in_=xvT_r[:, :, c0:c0 + cw])
            if ci == 0:
                nc.sync.dma_start(out=vones[:], in_=vones_d[:, :])
        bvr = const.tile([128, DH], F32)
        if with_bv:
            nc.sync.dma_start(out=bvr[:], in_=bvr_d[:, :])
        wo_sb = const.tile([128, MB * D], F16)
        nc.sync.dma_start(out=wo_sb[:], in_=woT_d[:, :])

        kT = const.tile([128, MB * nkp], F16)
        qT = const.tile([128, MB * nkp], F16)
        v_aug = const.tile([128, NKT * HC * VW], F16)
        ctx_dmaj = const.tile([128, MB * nkp], F16)
        vag = v_aug[:].rearrange("p (t h c) -> p t h c", t=NKT, h=HC)

        def emit_vones():
            # denominator ones column (validity per k-tile slot); emitted
            # as a filler so the DVE queue isn't blocked on the vones DMA
            nc.vector.tensor_copy(
                vag[:, :, :, DK:DK + 1],
                vones[:].rearrange("p (t h) -> p t h", t=NKT)[:, :, :, None])

        ps_s = top.enter_context(tc.tile_pool(name="ps_s", bufs=2, space="PSUM"))
        ps_c = top.enter_context(tc.tile_pool(name="ps_c", bufs=2, space="PSUM"))
        ps_w = top.enter_context(tc.tile_pool(name="ps_w", bufs=2, space="PSUM"))
        zpool = top.enter_context(tc.tile_pool(name="z", bufs=34))
        nqp = top.enter_context(tc.tile_pool(name="nq", bufs=12))
        rcp = top.enter_context(tc.tile_pool(name="rc", bufs=16))
        outsb = top.enter_context(tc.tile_pool(name="outsb", bufs=6))

        # ---------- emission units ----------
        def emit_kproj(m, c0, cw):
            ps = ps_w.tile([128, 512], F32, tag="psw", name="ps_k")
            for kc in range(PD):
                nc.tensor.matmul(
                    ps[:, 0:cw],
                    lhsT=wk_sb[:, (m * PD + kc) * 128:(m * PD + kc + 1) * 128],
                    rhs=xk_sb[:, nkp * kc + c0:nkp * kc + c0 + cw],
                    start=(kc == 0), stop=(kc == PD - 1))
            nc.vector.tensor_scalar_add(
                kT[:, nkp * m + c0:nkp * m + c0 + cw],
                ps[:, 0:cw], bk_sb[:, m:m + 1])

        def emit_qproj(j, m):
            c0, cw = JC[j]
            ps = ps_w.tile([128, 512], F32, tag="psw", name="ps_q")
            for kc in range(PD):
                nc.tensor.matmul(
                    ps[:, 0:cw],
                    lhsT=wq_sb[:, (m * PD + kc) * 128:(m * PD + kc + 1) * 128],
                    rhs=xq_sb[:, nkp * kc + c0:nkp * kc + c0 + cw],
                    start=(kc == 0), stop=(kc == PD - 1))
            nc.vector.tensor_scalar_add(
                qT[:, nkp * m + c0:nkp * m + c0 + cw],
                ps[:, 0:cw], bq_sb[:, m:m + 1])

        bvr_h = bvr[:].rearrange("p (h c) -> p h c", h=HC)

        def emit_vproj_group(g0):
            gs = min(2, NKT - g0)
            ps = ps_w.tile([128, 512], F32, tag="psw", name="ps_v")
            for s4 in range(gs):
                ii = g0 + s4
                for kc in range(PD):
                    nc.tensor.matmul(
                        ps[:, DH * s4:DH * (s4 + 1)],
                        lhsT=xv_sb[:, nkp * kc + 128 * ii:
                                   nkp * kc + 128 * (ii + 1)],
                        rhs=wv_sb[:, DH * kc:DH * (kc + 1)],
                        start=(kc == 0), stop=(kc == PD - 1))
            for s4 in range(gs):
                ii = g0 + s4
                dst = vag[:, ii, :, 0:DK]
                src = ps[:, DH * s4:DH * (s4 + 1)].rearrange(
                    "p (h c) -> p h c", h=HC)
                if with_bv:
                    nc.vector.tensor_add(dst, src, bvr_h)
                    nc.vector.tensor_scalar_mul(
                        dst, dst, vones[:, HC * ii:HC * ii + 1])
                else:
                    nc.vector.tensor_copy(dst, src)

        saved = {}
        zsave = {}

        # ctx accumulation: one region (t, qs) at a time — start_tensor_calc
        # marks the whole 2KB psum zero-region pending, so interleaved
        # accumulation groups within one bank clobber each other.  All 9
        # z tiles of a loop are kept alive and each region's 9 matmuls run
        # back-to-back (as a filler unit in the next loop).
        def emit_ctx_region(j, m, t, qs):
            c0, cw = JC[j]
            nq = cw // 128
            if (j, m) not in saved:
                saved[(j, m)] = ps_c.tile([128, 512], F32, tag="ctx",
                                          name=f"ctx_{j}_{m}")
            ctx = saved[(j, m)]
            zs = zsave[(j, m)]
            for ic in range(NKT):
                nc.tensor.matmul(
                    ctx[:, ((nq * t) + qs) * VW:((nq * t) + qs) * VW + VW],
                    lhsT=zs[ic][:, cw * t + 128 * qs:cw * t + 128 * (qs + 1)],
                    rhs=vag[:, ic, 2 * m + t, :],
                    start=(ic == 0), stop=(ic == NKT - 1))
            if t == 1 and qs == nq - 1:
                del zsave[(j, m)]

        def emit_norm(j, m, qs, split_eng=False):
            ctx = saved[(j, m)]
            c0, cw = JC[j]
            nq = cw // 128
            ctx_q = nqp.tile([128, 128], F16, tag="nq", name="ctx_q")
            for t in range(2):
                blk = (nq * t + qs) * VW
                r = rcp.tile([128, 1], F32, tag="rc", name="recip")
                nc.vector.reciprocal(r[:], ctx[:, blk + DK:blk + DK + 1])
                if split_eng and t == 1:
                    nc.scalar.mul(ctx_q[:, DK * t:DK * (t + 1)],
                                  ctx[:, blk:blk + DK], r[:])
                else:
                    nc.vector.tensor_scalar_mul(
                        ctx_q[:, DK * t:DK * (t + 1)],
                        ctx[:, blk:blk + DK], r[:])
            # [q, dh] -> [dh, q] via PE transpose (f16 psum out reuses the
            # psw slot size), DVE copyback; avoids the serialized HWDGE
            tp = ps_w.tile([128, 1024], F16, tag="psw", name="tp")
            nc.tensor.transpose(tp[:, 0:128], ctx_q[:], ident_h[:])
            nc.vector.tensor_copy(
                ctx_dmaj[:, nkp * m + c0 + 128 * qs:
                         nkp * m + c0 + 128 * (qs + 1)],
                tp[:, 0:128])

        osb_hold = {}
        stash = {}

        def emit_wo(j, dd, drain=False):
            c0, cw = JC[j]
            ps = ps_w.tile([128, 512], F32, tag="psw", name="ps_o")
            for m in range(MB):
                nc.tensor.matmul(
                    ps[:, 0:cw], lhsT=wo_sb[:, m * D + 128 * dd:
                                            m * D + 128 * (dd + 1)],
                    rhs=ctx_dmaj[:, nkp * m + c0:nkp * m + c0 + cw],
                    start=(m == 0), stop=(m == MB - 1))
            _wo_out(j, dd, ps, drain=drain)

        def emit_wo_p1(j, dd):
            c0, cw = JC[j]
            ps = ps_w.tile([128, 512], F32, tag="psw", name="ps_o1")
            nc.tensor.matmul(
                ps[:, 0:cw], lhsT=wo_sb[:, 128 * dd:128 * (dd + 1)],
                rhs=ctx_dmaj[:, c0:c0 + cw], start=True, stop=True)
            o0 = outsb.tile([128, JW], F16, tag="o0", bufs=10, name="o0")
            nc.vector.tensor_copy(o0[:, 0:cw], ps[:, 0:cw])
            stash[(j, dd)] = o0

        def emit_wo_p2(j, dd):
            c0, cw = JC[j]
            ps = ps_w.tile([128, 512], F32, tag="psw", name="ps_o2")
            nc.tensor.matmul(
                ps[:, 0:cw], lhsT=wo_sb[:, D + 128 * dd:D + 128 * (dd + 1)],
                rhs=ctx_dmaj[:, nkp + c0:nkp + c0 + cw],
                start=True, stop=False)
            nc.tensor.matmul(ps[:, 0:cw], lhsT=ident_h[:],
                             rhs=stash.pop((j, dd))[:, 0:cw],
                             start=False, stop=True)
            _wo_out(j, dd, ps, drain=True)

        def _wo_out(j, dd, ps, drain=False):
            # batch output DMAs in quads (HWDGE is a serialized 625ns/DMA
            # resource); the drain path uses eager pairs instead
            c0, cw = JC[j]
            grp = 2 if drain else 4
            if dd % grp == 0:
                osb_hold[j] = outsb.tile([128, grp * JW], F16, tag="osb",
                                         name="osb")
            osb = osb_hold[j]
            half = dd % grp
            eng = nc.scalar if (drain and dd % 2 == 1) else nc.vector
            if eng is nc.scalar:
                eng.copy(osb[:, JW * half:JW * half + cw], ps[:, 0:cw])
            else:
                eng.tensor_copy(osb[:, JW * half:JW * half + cw], ps[:, 0:cw])
            if half == grp - 1:
                nc.sync.dma_start(
                    out=outT_r[:, dd - grp + 1:dd + 1, c0:c0 + cw],
                    in_=osb[:].rearrange("p (g c) -> p g c",
                                         g=grp)[:, :, 0:cw])

        # ---------- schedule ----------
        # named FIFO of filler units; hard prerequisites enforced by need()
        fillers = deque()
        emitted = set()

        def push(name, fn):
            fillers.append((name, fn))

        def pop_filler():
            if fillers:
                name, fn = fillers.popleft()
                fn()
                emitted.add(name)

        def need(*names):
            while any(n not in emitted for n in names):
                assert fillers, f"missing prerequisite {names}"
                pop_filler()

        kproj_names = {0: [], 1: []}

        def attn_loop(j, m, igate=(), reserve=0):
            need(f"q{j}{m}", *kproj_names[m])
            c0, cw = JC[j]
            zs = zsave.setdefault((j, m), [])
            ig = list(igate)
            for i in range(NKT):
                while ig and ig[0][0] <= i:
                    need(ig.pop(0)[1])
                sps = ps_s.tile([128, 1024], F32, tag="sps", name="sps")
                for t in range(2):
                    nc.tensor.matmul(
                        sps[:, 512 * t:512 * t + cw],
                        lhsT=kT[64 * t:64 * (t + 1),
                                nkp * m + 128 * i:nkp * m + 128 * (i + 1)],
                        rhs=qT[64 * t:64 * (t + 1),
                               nkp * m + c0:nkp * m + c0 + cw],
                        start=True, stop=True)
                zt = zpool.tile([128, 2 * JW], F16, tag="z", name="zt")
                zv = zt[:].rearrange("p (b c) -> p b c", b=2)
                sv = sps[:].rearrange("p (b c) -> p b c", b=2)
                nc.scalar.activation(zv[:, :, 0:cw], sv[:, :, 0:cw],
                                     AF.Exp, scale=0.125)
                zs.append(zt)
                pop_filler()
                popped = 0
                while len(fillers) > (NKT - 1 - i) + reserve and popped < 4:
                    pop_filler()
                    popped += 1

        def push_ctx_norm(j, m, split_eng=False):
            # interleave: both ctx regions of a q-subtile, then its norm,
            # so transposes start early and spread out
            nq = JC[j][1] // 128
            for qs in range(nq):
                for t in range(2):
                    push(f"c{j}{m}{t}{qs}",
                         lambda j=j, m=m, t=t, qs=qs:
                         emit_ctx_region(j, m, t, qs))
            for qs in range(nq):
                push(f"n{j}{m}{qs}",
                     lambda j=j, m=m, qs=qs, s=split_eng:
                     emit_norm(j, m, qs, split_eng=s))

        # startup: kproj m0 chunk 0 + qproj (0,0) emitted directly; the
        # remaining kproj m0 chunks run as fillers inside loop 0, gated
        # per k-tile (their xk chunks stream in during the loop).
        emit_kproj(0, *KC[0])
        emit_qproj(0, 0)
        emitted.add("q00")
        igate0 = []
        for ci, (c0, cw) in enumerate(KC):
            if ci == 0:
                continue
            push(f"k0_{ci}", lambda c0=c0, cw=cw: emit_kproj(0, c0, cw))
            igate0.append((c0 // 128, f"k0_{ci}"))

        # j-outer loop order: (0,0),(0,1),(1,0),(1,1),... so wo(j) work
        # becomes available for the middle loops, not just the last ones
        assert NJ >= 3
        loops = [(j, m) for j in range(NJ) for m in range(2)]
        for li, (j, m) in enumerate(loops):
            if li == 0:
                for ci, (c0, cw) in enumerate(KC):
                    push(f"k1_{ci}",
                         lambda c0=c0, cw=cw: emit_kproj(1, c0, cw))
                    kproj_names[1].append(f"k1_{ci}")
            # ctx+norm of the previous loop (loop 0's delayed one extra
            # loop so the xv stream can land before the vproj units pop)
            if li >= 2:
                push_ctx_norm(*loops[li - 1])
                if li == 2:
                    push_ctx_norm(*loops[0])
            # qproj for the next loop, just in time
            if li + 1 < len(loops) and loops[li + 1] != (0, 0):
                nj, nm = loops[li + 1]
                push(f"q{nj}{nm}",
                     lambda nj=nj, nm=nm: emit_qproj(nj, nm))
            if li == 1:
                for g in range(0, NKT, 2):
                    push(f"v{g}", lambda g=g: emit_vproj_group(g))
                    if g == 0:
                        push("vones", emit_vones)
            # wo(x) once norm(x,1) is queued; last chunk via p1/p2 split
            if li >= 2 and loops[li - 1][1] == 1 and loops[li - 1][0] < NJ - 1:
                x = loops[li - 1][0]
                for dd in range(PD):
                    push(f"wo{x}_{dd}", lambda x=x, dd=dd: emit_wo(x, dd))
            attn_loop(j, m, igate0 if li == 0 else (),
                      reserve=6 if li == len(loops) - 2 else 0)
        # drain: last loop's ctx + norm, leftover fillers, wo p2
        push_ctx_norm(*loops[-1])
        while fillers:
            pop_filler()
        for dd in range(PD):
            emit_wo(NJ - 1, dd, drain=True)

    nc.compile()
    return nc


def _get_nc(nkp=NKP, with_bv=False):
    key = ("nc", nkp, with_bv)
    if key not in _cache:
        _cache[key] = _build_nc(nkp=nkp, with_bv=with_bv)
    return _cache[key]


def _blocked_wT(w, inner, m_major=False):
    """w [out_dim, in_dim] -> lhsT layout [128, (in/128)*out_dim] f16.
    kc-major (default): element [p, out*kc + 128*m + c] = w[128m+c, 128kc+p].
    m-major: element [p, (m*PD_in + kc)*128 + c] = same block, m outer."""
    wT = np.ascontiguousarray(w.T, np.float32)          # [in, out]
    nin, nout = wT.shape
    assert inner == nout
    blk = wT.reshape(nin // 128, 128, nout // 128, 128)   # [kc, p, m, c]
    if m_major:
        out = blk.transpose(1, 2, 0, 3)                   # [p, m, kc, c]
    else:
        out = blk.transpose(1, 0, 2, 3)                   # [p, kc, m, c]
    return np.ascontiguousarray(out.reshape(128, -1)).astype(np.float16)


def _shard_inputs(nkp, query, key, value, mask, wq, bq, wk, bk, wv, bv,
                  wo, bo):
    f16, f32 = np.float16, np.float32
    in_maps = []
    per_b = {}
    for b in range(B):
        maskb = np.ascontiguousarray(mask[b, 0]).astype(np.int32)
        idx = np.flatnonzero(maskb)
        nk = idx.size
        idx_pad = np.zeros(nkp, np.int64)
        idx_pad[:min(nk, nkp)] = idx[:nkp]
        keyc = np.asarray(key[b], f32)[idx_pad]
        valc = np.asarray(value[b], f32)[idx_pad]
        qryc = np.asarray(query[b], f32)[idx_pad]
        keyc[nk:] = 0.0
        valc[nk:] = 0.0
        qryc[nk:] = 0.0
        valid = np.zeros(nkp, f32)
        valid[:nk] = 1.0
        vones = np.repeat(valid.reshape(-1, 128).T[:, :, None], HC,
                          axis=2).reshape(128, -1)
        per_b[b] = {
            "xqT": np.ascontiguousarray(qryc.T).astype(f16),
            "xkT": np.ascontiguousarray(keyc.T).astype(f16),
            "xvT": np.ascontiguousarray(valc.T).astype(f16),
            "vones": np.ascontiguousarray(vones, f32),
        }
    for c in range(NCORES):
        b, r = c // 4, c % 4
        rows = slice(DH * r, DH * (r + 1))
        wq_r = np.asarray(wq, f32)[rows, :]
        wk_r = np.asarray(wk, f32)[rows, :]
        wv_r = np.asarray(wv, f32)[rows, :]
        wo_r = np.asarray(wo, f32)[:, rows]
        in_maps.append({
            **per_b[b],
            "wqT": _blocked_wT(wq_r, DH, m_major=True),
            "wkT": _blocked_wT(wk_r, DH, m_major=True),
            "wvT": _blocked_wT(wv_r, DH),
            "woT": _blocked_wT(wo_r, D),
            "bq": np.ascontiguousarray(
                np.asarray(bq, f32)[rows].reshape(MB, 128).T),
            "bk": np.ascontiguousarray(
                np.asarray(bk, f32)[rows].reshape(MB, 128).T),
            "bvr": np.ascontiguousarray(
                np.broadcast_to(np.asarray(bv, f32)[rows], (128, DH))),
        })
    return in_maps


def kernel(query, key, value, mask, wq, bq, wk, bk, wv, bv, wo, bo,
           _return_bench=False):
    mask = np.asarray(mask)
    nk_max = int(mask.reshape(B, -1).sum(1).max())
    nkp = NKP if nk_max <= NKP else ((nk_max + 127) // 128) * 128
    with_bv = bool(np.any(np.asarray(bv)))
    nc = _get_nc(nkp, with_bv)
    in_maps = _shard_inputs(nkp, np.asarray(query), np.asarray(key),
                            np.asarray(value), mask,
                            np.asarray(wq), np.asarray(bq),
                            np.asarray(wk), np.asarray(bk),
                            np.asarray(wv), np.asarray(bv),
                            np.asarray(wo), np.asarray(bo))
    trace = os.environ.get("KTRACE", "") == "1"
    res = run_bass_kernel_spmd(nc, in_maps, list(range(NCORES)), trace=trace)
    bo = np.asarray(bo, np.float32)
    out = np.empty((B, S, D), np.float32)
    for b in range(B):
        acc = res.results[4 * b]["outT"].astype(np.float32)
        for r in range(1, 4):
            acc += res.results[4 * b + r]["outT"].astype(np.float32)
        maskb = np.asarray(mask[b, 0])
        idx = np.flatnonzero(maskb)
        out[b, idx, :] = acc.T[:idx.size] + bo
        # masked queries: uniform softmax over ALL keys -> constant row
        qmask = maskb == 0
        if qmask.any():
            vmean = np.asarray(value[b], np.float32).mean(0)
            ctx_row = vmean @ np.asarray(wv, np.float32).T + np.asarray(
                bv, np.float32)
            const_row = ctx_row @ np.asarray(wo, np.float32).T + bo
            out[b, qmask, :] = const_row
    if _return_bench:
        return out, res
    return out


# revision 12
# speedup vs baseline: 1.1693x; 1.1143x over previous
"""MultiHeadedAttention Trainium2 kernel v3 (8 NeuronCores, SPMD).

Sharding: core c -> batch b = c//4, head-group r = c%4 (4 of 16 heads).

v3 vs v2 (137617 ns cost model):
  - QUERY compaction: the same mask that gates keys also gates queries
    (masked-query rows are a host-computed constant). The kernel now only
    processes the ~1046 valid queries (padded to 1152), cutting qproj /
    scores / exp / ctx / wo work by ~44%.
  - ctx in the "output-stationary" orientation: out[q,65] = z_sub^T @
    v_aug, 65-column matmuls (cost model charges by out free size), vs
    512-col in v2.  Softmax denominator still via the ones column.
  - normalization by per-partition scalar (reciprocal of the den column)
    fused into the PSUM->SBUF downcast copy on DVE; the [q,dh] -> [dh,q]
    layout flip for the output projection via PE transpose + DVE copyback
    (the shared HWDGE serializes DMA-xbar transposes at 625ns each).
  - PSUM accumulation groups kept strictly sequential per bank
    (start_tensor_calc marks the whole 2KB zero-region pending): all 9
    z tiles of a loop stay alive and each ctx region's 9 matmuls run
    back-to-back as filler units in the next loop.
  - named filler FIFO with explicit prerequisites; j-outer loop order;
    DMA stream ordered by deadline; quad-batched output DMAs.
  - all operand transposition/blocking on the HOST as in v2; f16 operands.

Per core (np = padded compacted token count, 1152 default):
    kT = wkT^T @ xkT            [256, np]   (dk-major, head-pair blocks)
    qT = wqT^T @ xqT            [256, np]   (compacted queries)
    v  = xvT^T @ wvT            [np, 256] -> v_aug [k-tile, head, 65]
    per (m, j-chunk of 384 q):
      sT = kT_h^T @ qT_h        [128k, 384q] per k-tile, 2 heads
      z  = exp(sT/8)            (f16, SBUF)
      ctx[q,65] += z_sub^T @ v_aug   (PSUM, 6 blocks of 65)
      ctx_q = ctx[:, :64] * recip(ctx[:, 64])   (DVE, f16)
      ctx_dmaj = transpose(ctx_q)               [dh, q] (PE+DVE)
    outT = woT^T @ ctx_dmaj     [D, np] f16
Host: out[b, valid] = sum_r outT_r^T + bo; masked rows = const row.

Self-contained: hardcodes B=2, S=2048, D=1024, H=16.
"""

import os
import numpy as np
from collections import deque
from contextlib import ExitStack

import concourse.bacc as bacc
import concourse.tile as tile
from concourse import mybir
from concourse.bass_utils import run_bass_kernel_spmd
from concourse.masks import make_identity

F32 = mybir.dt.float32
F16 = mybir.dt.float16
AF = mybir.ActivationFunctionType

B, S, D, H = 2, 2048, 1024, 16
DK = 64                      # head dim
HC = 4                       # heads per core
DH = HC * DK                 # 256, local head width
MB = DH // 128               # 2 partition blocks of local head dims
PD = D // 128                # 8 d-blocks
JW = 384                     # q chunk width (<=512 psum bank, 3 qsubs)
VW = DK + 1                  # 65: v columns + denominator ones column
NCORES = 8

_cache = {}
_last_nc = None
NKP = 1152                   # padded compacted token count (nk_max=1046)


def _chunks(n, step, first=None):
    c0 = 0
    while c0 < n:
        cw = min(first if (first and c0 == 0) else step, n - c0)
        yield c0, cw
        c0 += cw


def _build_nc(nkp=NKP, nvq=None, with_bv=False):
    NKT = nkp // 128
    if nvq is None:
        nvq = nkp
    # (c0, cw, vw): cw = layout width (128-mult), vw = valid width used
    # by scores/exp/qproj/wo (queries beyond nvq are padding)
    JC = [(c0, cw, max(1, min(cw, nvq - c0)))
          for c0, cw in _chunks(nkp, JW)]
    NJ = len(JC)
    nc = bacc.Bacc("TRN2", target_bir_lowering=False, debug=False,
                   num_devices=NCORES)

    xqT_d = nc.dram_tensor("xqT", [D, nkp], F16, kind="ExternalInput").ap()
    xkT_d = nc.dram_tensor("xkT", [D, nkp], F16, kind="ExternalInput").ap()
    xvT_d = nc.dram_tensor("xvT", [D, nkp], F16, kind="ExternalInput").ap()
    wqT_d = nc.dram_tensor("wqT", [128, PD * DH], F16, kind="ExternalInput").ap()
    wkT_d = nc.dram_tensor("wkT", [128, PD * DH], F16, kind="ExternalInput").ap()
    wvT_d = nc.dram_tensor("wvT", [128, PD * DH], F16, kind="ExternalInput").ap()
    woT_d = nc.dram_tensor("woT", [128, MB * D], F16, kind="ExternalInput").ap()
    bq_d = nc.dram_tensor("bq", [128, MB], F32, kind="ExternalInput").ap()
    bk_d = nc.dram_tensor("bk", [128, MB], F32, kind="ExternalInput").ap()
    bvr_d = nc.dram_tensor("bvr", [128, DH], F32, kind="ExternalInput").ap()
    vones_d = nc.dram_tensor("vones", [128, NKT * HC], F32,
                             kind="ExternalInput").ap()
    outT_d = nc.dram_tensor("outT", [D, nkp], F16, kind="ExternalOutput").ap()

    xqT_r = xqT_d.rearrange("(a p) c -> p a c", p=128)   # [128, PD, nkp]
    xkT_r = xkT_d.rearrange("(a p) c -> p a c", p=128)
    xvT_r = xvT_d.rearrange("(a p) c -> p a c", p=128)
    outT_r = outT_d.rearrange("(g p) c -> p g c", p=128)  # [128, 8, nkp]

    with tile.TileContext(nc) as tc, ExitStack() as top:
        const = top.enter_context(tc.tile_pool(name="const", bufs=1))
        ident_f = const.tile([128, 128], F32)
        make_identity(nc, ident_f)
        ident_h = const.tile([128, 128], F16)
        nc.vector.tensor_copy(ident_h[:], ident_f[:])

        # --- input DMAs (merged 3D APs), ordered by deadline ---
        # DMA transfers serialize on the (single-slot) DMA_ENGINES resource,
        # so the stream order IS the arrival order.
        KC = [(c0, cw, max(1, min(cw, nvq - c0)))
              for c0, cw in _chunks(nkp, 512, first=256)]  # kproj/xk chunks
        wk_sb = const.tile([128, PD * DH], F16)
        nc.sync.dma_start(out=wk_sb[:, 0:PD * 128],
                          in_=wkT_d[:, 0:PD * 128])
        bk_sb = const.tile([128, MB], F32)
        nc.sync.dma_start(out=bk_sb[:], in_=bk_d[:, :])
        xk_sb = const.tile([128, PD * nkp], F16)
        xk_v = xk_sb[:].rearrange("p (a c) -> p a c", a=PD)
        nc.sync.dma_start(out=xk_v[:, :, 0:KC[0][2]],
                          in_=xkT_r[:, :, 0:KC[0][2]])
        bq_sb = const.tile([128, MB], F32)
        nc.sync.dma_start(out=bq_sb[:], in_=bq_d[:, :])
        wq_sb = const.tile([128, PD * DH], F16)
        nc.sync.dma_start(out=wq_sb[:, 0:PD * 128],
                          in_=wqT_d[:, 0:PD * 128])
        xq_sb = const.tile([128, PD * nkp], F16)
        xq_v = xq_sb[:].rearrange("p (a c) -> p a c", a=PD)
        nc.sync.dma_start(out=xq_v[:, :, 0:JW], in_=xqT_r[:, :, 0:JW])
        for c0, cw, vw in KC[1:]:
            nc.sync.dma_start(out=xk_v[:, :, c0:c0 + vw],
                              in_=xkT_r[:, :, c0:c0 + vw])
        nc.sync.dma_start(out=wk_sb[:, PD * 128:2 * PD * 128],
                          in_=wkT_d[:, PD * 128:2 * PD * 128])
        nc.sync.dma_start(out=wq_sb[:, PD * 128:2 * PD * 128],
                          in_=wqT_d[:, PD * 128:2 * PD * 128])
        if nkp > JW:
            nc.sync.dma_start(out=xq_v[:, :, JW:min(2 * JW, nkp)],
                              in_=xqT_r[:, :, JW:min(2 * JW, nkp)])
        wv_sb = const.tile([128, PD * DH], F16)
        nc.sync.dma_start(out=wv_sb[:], in_=wvT_d[:, :])
        for c0, cw in _chunks(nkp, JW):
            if c0 < 2 * JW:
                continue
            vw = max(1, min(cw, nvq - c0))
            nc.sync.dma_start(out=xq_v[:, :, c0:c0 + vw],
                              in_=xqT_r[:, :, c0:c0 + vw])
        xv_sb = const.tile([128, PD * nkp], F16)
        xv_v = xv_sb[:].rearrange("p (a c) -> p a c", a=PD)
        vones = const.tile([128, NKT * HC], F32)
        for ci, (c0, cw) in enumerate(_chunks(nkp, 512, first=256)):
            nc.sync.dma_start(out=xv_v[:, :, c0:c0 + cw],
                              in_=xvT_r[:, :, c0:c0 + cw])
            if ci == 0:
                nc.sync.dma_start(out=vones[:], in_=vones_d[:, :])
        bvr = const.tile([128, DH], F32)
        if with_bv:
            nc.sync.dma_start(out=bvr[:], in_=bvr_d[:, :])
        wo_sb = const.tile([128, MB * D], F16)
        nc.sync.dma_start(out=wo_sb[:], in_=woT_d[:, :])

        kT = const.tile([128, MB * nkp], F16)
        qT = const.tile([128, MB * nkp], F16)
        if nvq < nkp:
            # kproj skips the padded key columns; zero them once so the
            # score tiles read finite values (contributions are nulled by
            # the zeroed v_aug rows / vones column)
            for m in range(MB):
                nc.vector.memset(kT[:, nkp * m + nvq:nkp * (m + 1)], 0.0)
        v_aug = const.tile([128, NKT * HC * VW], F16)
        ctx_dmaj = const.tile([128, MB * nkp], F16)
        vag = v_aug[:].rearrange("p (t h c) -> p t h c", t=NKT, h=HC)

        def emit_vones():
            # denominator ones column (validity per k-tile slot); emitted
            # as a filler so the DVE queue isn't blocked on the vones DMA
            nc.vector.tensor_copy(
                vag[:, :, :, DK:DK + 1],
                vones[:].rearrange("p (t h) -> p t h", t=NKT)[:, :, :, None])

        ps_s = top.enter_context(tc.tile_pool(name="ps_s", bufs=2, space="PSUM"))
        ps_c = top.enter_context(tc.tile_pool(name="ps_c", bufs=2, space="PSUM"))
        ps_w = top.enter_context(tc.tile_pool(name="ps_w", bufs=2, space="PSUM"))
        zpool = top.enter_context(tc.tile_pool(name="z", bufs=34))
        nqp = top.enter_context(tc.tile_pool(name="nq", bufs=12))
        rcp = top.enter_context(tc.tile_pool(name="rc", bufs=16))
        outsb = top.enter_context(tc.tile_pool(name="outsb", bufs=6))

        # ---------- emission units ----------
        def emit_kproj(m, c0, cw):
            cw = max(1, min(cw, nvq - c0))
            ps = ps_w.tile([128, 512], F32, tag="psw", name="ps_k")
            for kc in range(PD):
                nc.tensor.matmul(
                    ps[:, 0:cw],
                    lhsT=wk_sb[:, (m * PD + kc) * 128:(m * PD + kc + 1) * 128],
                    rhs=xk_sb[:, nkp * kc + c0:nkp * kc + c0 + cw],
                    start=(kc == 0), stop=(kc == PD - 1))
            nc.vector.tensor_scalar_add(
                kT[:, nkp * m + c0:nkp * m + c0 + cw],
                ps[:, 0:cw], bk_sb[:, m:m + 1])

        def emit_qproj(j, m):
            c0, cw, vw = JC[j]
            ps = ps_w.tile([128, 512], F32, tag="psw", name="ps_q")
            for kc in range(PD):
                nc.tensor.matmul(
                    ps[:, 0:vw],
                    lhsT=wq_sb[:, (m * PD + kc) * 128:(m * PD + kc + 1) * 128],
                    rhs=xq_sb[:, nkp * kc + c0:nkp * kc + c0 + vw],
                    start=(kc == 0), stop=(kc == PD - 1))
            nc.vector.tensor_scalar_add(
                qT[:, nkp * m + c0:nkp * m + c0 + vw],
                ps[:, 0:vw], bq_sb[:, m:m + 1])

        bvr_h = bvr[:].rearrange("p (h c) -> p h c", h=HC)

        def emit_vproj_group(g0):
            gs = min(2, NKT - g0)
            ps = ps_w.tile([128, 512], F32, tag="psw", name="ps_v")
            for s4 in range(gs):
                ii = g0 + s4
                for kc in range(PD):
                    nc.tensor.matmul(
                        ps[:, DH * s4:DH * (s4 + 1)],
                        lhsT=xv_sb[:, nkp * kc + 128 * ii:
                                   nkp * kc + 128 * (ii + 1)],
                        rhs=wv_sb[:, DH * kc:DH * (kc + 1)],
                        start=(kc == 0), stop=(kc == PD - 1))
            for s4 in range(gs):
                ii = g0 + s4
                dst = vag[:, ii, :, 0:DK]
                src = ps[:, DH * s4:DH * (s4 + 1)].rearrange(
                    "p (h c) -> p h c", h=HC)
                if with_bv:
                    nc.vector.tensor_add(dst, src, bvr_h)
                    nc.vector.tensor_scalar_mul(
                        dst, dst, vones[:, HC * ii:HC * ii + 1])
                else:
                    nc.vector.tensor_copy(dst, src)

        saved = {}
        zsave = {}

        # ctx accumulation: one region (t, qs) at a time — start_tensor_calc
        # marks the whole 2KB psum zero-region pending, so interleaved
        # accumulation groups within one bank clobber each other.  All 9
        # z tiles of a loop are kept alive and each region's 9 matmuls run
        # back-to-back (as a filler unit in the next loop).
        def emit_ctx_region(j, m, t, qs):
            c0, cw, vw = JC[j]
            nq = cw // 128
            qw = min(128, vw - 128 * qs)
            if (j, m) not in saved:
                saved[(j, m)] = ps_c.tile([128, 512], F32, tag="ctx",
                                          name=f"ctx_{j}_{m}")
            ctx = saved[(j, m)]
            zs = zsave[(j, m)]
            for ic in range(NKT):
                nc.tensor.matmul(
                    ctx[0:qw, ((nq * t) + qs) * VW:((nq * t) + qs) * VW + VW],
                    lhsT=zs[ic][:, cw * t + 128 * qs:
                                cw * t + 128 * qs + qw],
                    rhs=vag[:, ic, 2 * m + t, :],
                    start=(ic == 0), stop=(ic == NKT - 1))
            if t == 1 and qs == nq - 1:
                del zsave[(j, m)]

        def emit_norm(j, m, qs, split_eng=False):
            ctx = saved[(j, m)]
            c0, cw, vw = JC[j]
            nq = cw // 128
            qw = min(128, vw - 128 * qs)
            ctx_q = nqp.tile([128, 128], F16, tag="nq", name="ctx_q")
            for t in range(2):
                blk = (nq * t + qs) * VW
                r = rcp.tile([128, 1], F32, tag="rc", name="recip")
                nc.vector.reciprocal(r[0:qw],
                                     ctx[0:qw, blk + DK:blk + DK + 1])
                if split_eng and t == 1:
                    nc.scalar.mul(ctx_q[0:qw, DK * t:DK * (t + 1)],
                                  ctx[0:qw, blk:blk + DK], r[0:qw])
                else:
                    nc.vector.tensor_scalar_mul(
                        ctx_q[0:qw, DK * t:DK * (t + 1)],
                        ctx[0:qw, blk:blk + DK], r[0:qw])
            # [q, dh] -> [dh, q] via PE transpose (f16 psum out reuses the
            # psw slot size), DVE copyback; avoids the serialized HWDGE
            tp = ps_w.tile([128, 1024], F16, tag="psw", name="tp")
            nc.tensor.transpose(tp[:, 0:128], ctx_q[:], ident_h[:])
            nc.vector.tensor_copy(
                ctx_dmaj[:, nkp * m + c0 + 128 * qs:
                         nkp * m + c0 + 128 * (qs + 1)],
                tp[:, 0:128])

        osb_hold = {}
        stash = {}

        def emit_wo(j, dd, drain=False):
            c0, cw, vw = JC[j]
            ps = ps_w.tile([128, 512], F32, tag="psw", name="ps_o")
            for m in range(MB):
                nc.tensor.matmul(
                    ps[:, 0:vw], lhsT=wo_sb[:, m * D + 128 * dd:
                                            m * D + 128 * (dd + 1)],
                    rhs=ctx_dmaj[:, nkp * m + c0:nkp * m + c0 + vw],
                    start=(m == 0), stop=(m == MB - 1))
            _wo_out(j, dd, ps, drain=drain)

        def emit_wo_p1(j, dd):
            c0, cw = JC[j]
            ps = ps_w.tile([128, 512], F32, tag="psw", name="ps_o1")
            nc.tensor.matmul(
                ps[:, 0:cw], lhsT=wo_sb[:, 128 * dd:128 * (dd + 1)],
                rhs=ctx_dmaj[:, c0:c0 + cw], start=True, stop=True)
            o0 = outsb.tile([128, JW], F16, tag="o0", bufs=10, name="o0")
            nc.vector.tensor_copy(o0[:, 0:cw], ps[:, 0:cw])
            stash[(j, dd)] = o0

        def emit_wo_p2(j, dd):
            c0, cw = JC[j]
            ps = ps_w.tile([128, 512], F32, tag="psw", name="ps_o2")
            nc.tensor.matmul(
                ps[:, 0:cw], lhsT=wo_sb[:, D + 128 * dd:D + 128 * (dd + 1)],
                rhs=ctx_dmaj[:, nkp + c0:nkp + c0 + cw],
                start=True, stop=False)
            nc.tensor.matmul(ps[:, 0:cw], lhsT=ident_h[:],
                             rhs=stash.pop((j, dd))[:, 0:cw],
                             start=False, stop=True)
            _wo_out(j, dd, ps, drain=True)

        def _wo_out(j, dd, ps, drain=False):
            # batch output DMAs in quads (HWDGE is a serialized 625ns/DMA
            # resource); the drain path uses eager pairs instead
            c0, cw, vw = JC[j]
            cw = vw
            grp = 2 if drain else 4
            if dd % grp == 0:
                osb_hold[j] = outsb.tile([128, grp * JW], F16, tag="osb",
                                         name="osb")
            osb = osb_hold[j]
            half = dd % grp
            eng = nc.scalar if (drain and dd % 2 == 1) else nc.vector
            if eng is nc.scalar:
                eng.copy(osb[:, JW * half:JW * half + cw], ps[:, 0:cw])
            else:
                eng.tensor_copy(osb[:, JW * half:JW * half + cw], ps[:, 0:cw])
            if half == grp - 1:
                nc.sync.dma_start(
                    out=outT_r[:, dd - grp + 1:dd + 1, c0:c0 + cw],
                    in_=osb[:].rearrange("p (g c) -> p g c",
                                         g=grp)[:, :, 0:cw])

        # ---------- schedule ----------
        # named FIFO of filler units; hard prerequisites enforced by need()
        fillers = deque()
        emitted = set()

        def push(name, fn):
            fillers.append((name, fn))

        def pop_filler():
            if fillers:
                name, fn = fillers.popleft()
                fn()
                emitted.add(name)

        def need(*names):
            while any(n not in emitted for n in names):
                assert fillers, f"missing prerequisite {names}"
                pop_filler()

        kproj_names = {0: [], 1: []}

        def attn_loop(j, m, igate=(), reserve=0):
            need(f"q{j}{m}", *kproj_names[m])
            c0, cw, vw = JC[j]
            zs = zsave.setdefault((j, m), [])
            ig = list(igate)
            for i in range(NKT):
                while ig and ig[0][0] <= i:
                    need(ig.pop(0)[1])
                sps = ps_s.tile([128, 1024], F32, tag="sps", name="sps")
                for t in range(2):
                    nc.tensor.matmul(
                        sps[:, 512 * t:512 * t + vw],
                        lhsT=kT[64 * t:64 * (t + 1),
                                nkp * m + 128 * i:nkp * m + 128 * (i + 1)],
                        rhs=qT[64 * t:64 * (t + 1),
                               nkp * m + c0:nkp * m + c0 + vw],
                        start=True, stop=True)
                zt = zpool.tile([128, 2 * JW], F16, tag="z", name="zt")
                zv = zt[:].rearrange("p (b c) -> p b c", b=2)
                sv = sps[:].rearrange("p (b c) -> p b c", b=2)
                nc.scalar.activation(zv[:, :, 0:vw], sv[:, :, 0:vw],
                                     AF.Exp, scale=0.125)
                zs.append(zt)
                pop_filler()
                popped = 0
                while len(fillers) > (NKT - 1 - i) + reserve and popped < 4:
                    pop_filler()
                    popped += 1

        def push_regions(j, m):
            nq = (JC[j][2] + 127) // 128
            for qs in range(nq):
                for t in range(2):
                    push(f"c{j}{m}{t}{qs}",
                         lambda j=j, m=m, t=t, qs=qs:
                         emit_ctx_region(j, m, t, qs))

        def push_norms(j, m, split_eng=False):
            nq = (JC[j][2] + 127) // 128
            for qs in range(nq):
                push(f"n{j}{m}{qs}",
                     lambda j=j, m=m, qs=qs, s=split_eng:
                     emit_norm(j, m, qs, split_eng=s))

        def push_ctx_norm(j, m, split_eng=False):
            push_regions(j, m)
            push_norms(j, m, split_eng)

        # startup: kproj m0 chunk 0 + qproj (0,0) emitted directly; the
        # remaining kproj m0 chunks run as fillers inside loop 0, gated
        # per k-tile (their xk chunks stream in during the loop).
        emit_kproj(0, *KC[0][:2])
        emit_qproj(0, 0)
        emitted.add("q00")
        igate0 = []
        for ci, (c0, cw, vw) in enumerate(KC):
            if ci == 0:
                continue
            push(f"k0_{ci}", lambda c0=c0, cw=cw: emit_kproj(0, c0, cw))
            igate0.append((c0 // 128, f"k0_{ci}"))

        # j-outer loop order: (0,0),(0,1),(1,0),(1,1),... so wo(j) work
        # becomes available for the middle loops, not just the last ones
        assert NJ >= 3
        loops = [(j, m) for j in range(NJ) for m in range(2)]
        for li, (j, m) in enumerate(loops):
            if li == 0:
                for ci, (c0, cw, vw) in enumerate(KC):
                    push(f"k1_{ci}",
                         lambda c0=c0, cw=cw: emit_kproj(1, c0, cw))
                    kproj_names[1].append(f"k1_{ci}")
            # ctx+norm of the previous loop (loop 0's delayed one extra
            # loop so the xv stream can land before the vproj units pop)
            if li >= 2:
                if li >= 3:
                    push_norms(*loops[li - 2])
                if li == 3:
                    push_norms(*loops[0])
                push_regions(*loops[li - 1])
                if li == 2:
                    push_regions(*loops[0])
            # qproj for the next loop, just in time
            if li + 1 < len(loops) and loops[li + 1] != (0, 0):
                nj, nm = loops[li + 1]
                push(f"q{nj}{nm}",
                     lambda nj=nj, nm=nm: emit_qproj(nj, nm))
            if li == 1:
                for g in range(0, NKT, 2):
                    push(f"v{g}", lambda g=g: emit_vproj_group(g))
                    if g == 0:
                        push("vones", emit_vones)
            # wo(x) once norm(x,1) is queued; last chunk via p1/p2 split
            if li >= 3 and loops[li - 2][1] == 1 and loops[li - 2][0] < NJ - 1:
                x = loops[li - 2][0]
                for dd in range(PD):
                    push(f"wo{x}_{dd}", lambda x=x, dd=dd: emit_wo(x, dd))
            attn_loop(j, m, igate0 if li == 0 else (),
                      reserve=(12 if li == len(loops) - 1 else
                               10 if li == len(loops) - 2 else
                               6 if li >= 2 else 0))
        # drain: last loop's ctx + norm, leftover fillers, wo p2
        push_norms(*loops[-2])
        push_ctx_norm(*loops[-1])
        while fillers:
            pop_filler()
        for dd in range(PD):
            emit_wo(NJ - 1, dd, drain=True)

    nc.compile()
    return nc


def _get_nc(nkp=NKP, nvq=None, with_bv=False):
    key = ("nc", nkp, nvq, with_bv)
    if key not in _cache:
        _cache[key] = _build_nc(nkp=nkp, nvq=nvq, with_bv=with_bv)
    return _cache[key]


def _blocked_wT(w, inner, m_major=False):
    """w [out_dim, in_dim] -> lhsT layout [128, (in/128)*out_dim] f16.
    kc-major (default): element [p, out*kc + 128*m + c] = w[128m+c, 128kc+p].
    m-major: element [p, (m*PD_in + kc)*128 + c] = same block, m outer."""
    wT = np.ascontiguousarray(w.T, np.float32)          # [in, out]
    nin, nout = wT.shape
    assert inner == nout
    blk = wT.reshape(nin // 128, 128, nout // 128, 128)   # [kc, p, m, c]
    if m_major:
        out = blk.transpose(1, 2, 0, 3)                   # [p, m, kc, c]
    else:
        out = blk.transpose(1, 0, 2, 3)                   # [p, kc, m, c]
    return np.ascontiguousarray(out.reshape(128, -1)).astype(np.float16)


def _shard_inputs(nkp, query, key, value, mask, wq, bq, wk, bk, wv, bv,
                  wo, bo):
    f16, f32 = np.float16, np.float32
    in_maps = []
    per_b = {}
    for b in range(B):
        maskb = np.ascontiguousarray(mask[b, 0]).astype(np.int32)
        idx = np.flatnonzero(maskb)
        nk = idx.size
        idx_pad = np.zeros(nkp, np.int64)
        idx_pad[:min(nk, nkp)] = idx[:nkp]
        keyc = np.asarray(key[b], f32)[idx_pad]
        valc = np.asarray(value[b], f32)[idx_pad]
        qryc = np.asarray(query[b], f32)[idx_pad]
        keyc[nk:] = 0.0
        valc[nk:] = 0.0
        qryc[nk:] = 0.0
        valid = np.zeros(nkp, f32)
        valid[:nk] = 1.0
        vones = np.repeat(valid.reshape(-1, 128).T[:, :, None], HC,
                          axis=2).reshape(128, -1)
        per_b[b] = {
            "xqT": np.ascontiguousarray(qryc.T).astype(f16),
            "xkT": np.ascontiguousarray(keyc.T).astype(f16),
            "xvT": np.ascontiguousarray(valc.T).astype(f16),
            "vones": np.ascontiguousarray(vones, f32),
        }
    for c in range(NCORES):
        b, r = c // 4, c % 4
        rows = slice(DH * r, DH * (r + 1))
        wq_r = np.asarray(wq, f32)[rows, :]
        wk_r = np.asarray(wk, f32)[rows, :]
        wv_r = np.asarray(wv, f32)[rows, :]
        wo_r = np.asarray(wo, f32)[:, rows]
        in_maps.append({
            **per_b[b],
            "wqT": _blocked_wT(wq_r, DH, m_major=True),
            "wkT": _blocked_wT(wk_r, DH, m_major=True),
            "wvT": _blocked_wT(wv_r, DH),
            "woT": _blocked_wT(wo_r, D),
            "bq": np.ascontiguousarray(
                np.asarray(bq, f32)[rows].reshape(MB, 128).T),
            "bk": np.ascontiguousarray(
                np.asarray(bk, f32)[rows].reshape(MB, 128).T),
            "bvr": np.ascontiguousarray(
                np.broadcast_to(np.asarray(bv, f32)[rows], (128, DH))),
        })
    return in_maps


def kernel(query, key, value, mask, wq, bq, wk, bk, wv, bv, wo, bo,
           _return_bench=False):
    mask = np.asarray(mask)
    nk_max = int(mask.reshape(B, -1).sum(1).max())
    nkp = NKP if nk_max <= NKP else ((nk_max + 127) // 128) * 128
    with_bv = bool(np.any(np.asarray(bv)))
    nc = _get_nc(nkp, nk_max, with_bv)
    global _last_nc
    _last_nc = nc
    in_maps = _shard_inputs(nkp, np.asarray(query), np.asarray(key),
                            np.asarray(value), mask,
                            np.asarray(wq), np.asarray(bq),
                            np.asarray(wk), np.asarray(bk),
                            np.asarray(wv), np.asarray(bv),
                            np.asarray(wo), np.asarray(bo))
    trace = os.environ.get("KTRACE", "") == "1"
    res = run_bass_kernel_spmd(nc, in_maps, list(range(NCORES)), trace=trace)
    bo = np.asarray(bo, np.float32)
    out = np.empty((B, S, D), np.float32)
    for b in range(B):
        acc = res.results[4 * b]["outT"].astype(np.float32)
        for r in range(1, 4):
            acc += res.results[4 * b + r]["outT"].astype(np.float32)
        maskb = np.asarray(mask[b, 0])
        idx = np.flatnonzero(maskb)
        out[b, idx, :] = acc.T[:idx.size] + bo
        # masked queries: uniform softmax over ALL keys -> constant row
        qmask = maskb == 0
        if qmask.any():
            vmean = np.asarray(value[b], np.float32).mean(0)
            ctx_row = vmean @ np.asarray(wv, np.float32).T + np.asarray(
                bv, np.float32)
            const_row = ctx_row @ np.asarray(wo, np.float32).T + bo
            out[b, qmask, :] = const_row
    if _return_bench:
        return out, res
    return out
